# revision 1
# baseline (speedup 1.0000x reference)
"""Trainium2 Bass kernel for nn_ComplexFFTRadNet (complex CNN, 4 cconv+BN+ReLU
layers, |.| magnitude, two 3x3 conv heads, sigmoid on cls channel).

Sharding: 8 NeuronCores = batch(4) x H-halves(2). Each core computes 64 output
rows of one image. Bottom halves are vertically FLIPPED on the host (input rows
and conv-kernel dy both reversed) so that every core runs the identical SPMD
program: global image edge at local top, 5 rows of neighbor halo at local
bottom. BatchNorm statistics (training-style, over N,H,W) are computed locally
per channel with bn_stats/bn_aggr over each core's owned 64 rows and combined
with a tiny AllReduce per layer.

Convolution = 9 shifted-window matmuls accumulated in PSUM; channels on the
partition (contraction) axis; activations stored row-major [C, H, W+2] with
zero pad columns so all 9 taps are contiguous-offset reads of one SBUF tile.
Real/imag are stacked along channels, so a complex conv is one real conv with
the block weight matrix [[wr, -wi], [wi, wr]].
"""
import os
import sys
import numpy as np
from contextlib import ExitStack

sys.path.insert(0, "/opt/trn_rl_repo")

from concourse import bass, bass_utils, tile, mybir, bacc  # noqa: E402

try:
    import ml_dtypes
    _BF16 = ml_dtypes.bfloat16
except Exception:  # pragma: no cover
    _BF16 = None

N_CORES = 8
H, W = 128, 224
WB = W + 2          # padded width
OWN = 64            # owned rows per core
R = 8               # output rows per chunk
CNT_LOCAL = float(OWN * W)          # elements per channel per core
CNT_TOTAL = float(4 * H * W)        # elements per channel globally
BN_EPS = 1e-5

# matmul/storage dtype: "f32" (exact) or "bf16" (4x faster PE, ~0.5% err)
DT_MODE = os.environ.get("KERNEL_DT", "bf16")

# layer table: (n_kg_load, kg_ch, K, n_kg_mm, n_mg, M_total, H_in_data, H_out)
#   L1: x[256] -> stacked 288 (yr144,yi144), kgroups 2x128, mgroups 3x96
#   L2: 288 -> 192, kgroups 3x96, mgroups 2x96
#   L3, L4: 192 -> 192, kgroups 2x96, mgroups 2x96
#   L5 head: mag[96] -> 3
LAYERS = [
    dict(K=128, n_kg=2, Cin=256, n_mg=3, Mg=96, Mtot=288, Hin=69, Hout=68),
    dict(K=96, n_kg=3, Cin=288, n_mg=2, Mg=96, Mtot=192, Hin=68, Hout=67),
    dict(K=96, n_kg=2, Cin=192, n_mg=2, Mg=96, Mtot=192, Hin=67, Hout=66),
    dict(K=96, n_kg=2, Cin=192, n_mg=2, Mg=96, Mtot=192, Hin=66, Hout=65),
    dict(K=96, n_kg=2, Cin=192, n_mg=1, Mg=3, Mtot=3, Hin=65, Hout=64),
]

_nc_cache = {}


def _dt(mode):
    return mybir.dt.bfloat16 if mode == "bf16" else mybir.dt.float32


def _npdt(mode):
    return _BF16 if mode == "bf16" else np.float32


def build_program(mode):
    if mode in _nc_cache:
        return _nc_cache[mode]
    DT = _dt(mode)
    F32 = mybir.dt.float32
    R = 16 if mode == "bf16" else 8  # chunk rows (SBUF-limited for f32)
    nc = bacc.Bacc("TRN2", target_bir_lowering=False, debug=False,
                   num_devices=N_CORES)

    # ---- external I/O ----
    x_ext = nc.dram_tensor("x", [256, 70, WB], DT, kind="ExternalInput").ap()
    w_ext = []
    for li, L in enumerate(LAYERS):
        w_ext.append(nc.dram_tensor(
            f"w{li + 1}", [L["K"], 9, L["n_kg"] if li != 4 else 1, L["Mtot"]],
            DT, kind="ExternalInput").ap())
    gb_ext = []
    for li in range(4):
        gb_ext.append(nc.dram_tensor(
            f"gb{li + 1}", [LAYERS[li]["Mtot"], 2], F32,
            kind="ExternalInput").ap())
    hb_ext = nc.dram_tensor("hb", [3, 1], F32, kind="ExternalInput").ap()
    out_ext = nc.dram_tensor("out", [3, OWN, W], F32,
                             kind="ExternalOutput").ap()

    FLAT = (R + 2) * WB + 2  # flat in-tile size (1 lead + rows+2 + 1 tail)

    with tile.TileContext(nc) as tc, ExitStack() as ctx:
        wpool = ctx.enter_context(tc.tile_pool(name="wts", bufs=2))
        inpool = ctx.enter_context(tc.tile_pool(name="inp", bufs=2))
        stpool = ctx.enter_context(tc.tile_pool(name="stage", bufs=3))
        pspool = ctx.enter_context(tc.tile_pool(name="ps", bufs=8, space="PSUM"))
        stats = ctx.enter_context(tc.tile_pool(name="stats", bufs=1))
        small = ctx.enter_context(tc.tile_pool(name="small", bufs=4))
        stp = ctx.enter_context(tc.tile_pool(name="stv", bufs=2))
        dram = ctx.enter_context(tc.tile_pool(name="dram", bufs=1, space="DRAM"))

        # constants
        eps_t = small.tile([128, 1], F32, tag="eps")
        nc.vector.memset(eps_t[:], BN_EPS)
        hb_t = small.tile([3, 1], F32, tag="hb")
        nc.sync.dma_start(out=hb_t[:], in_=hb_ext)

        # per-layer weights resident whole kernel
        w_t = []
        for li, L in enumerate(LAYERS):
            nkg = L["n_kg"] if li != 4 else 1
            t = wpool.tile([L["K"], 9, nkg, L["Mtot"]], DT, tag="w",
                           name=f"wt{li}")
            nc.sync.dma_start(out=t[:], in_=w_ext[li])
            w_t.append(t)

        # DRAM spill buffers for layer outputs (raw conv out, pre-BN)
        y_dram = []
        for li in range(4):
            L = LAYERS[li]
            y_dram.append(dram.tile([L["Mtot"], L["Hout"], WB], DT,
                                    tag=f"y{li}", name=f"y{li}"))
        cc_in = [dram.tile([LAYERS[li]["Mtot"], 2], F32, tag=f"cci{li}",
                           name=f"cci{li}")
                 for li in range(4)]
        cc_out = [dram.tile([LAYERS[li]["Mtot"], 2], F32, tag=f"cco{li}",
                            name=f"cco{li}")
                  for li in range(4)]

        st_cur = None  # list per kgroup of [96,2] tiles (s=col0, t=col1)

        for li, L in enumerate(LAYERS):
            K, Mg, Mtot, Hout, Hin = L["K"], L["Mg"], L["Mtot"], L["Hout"], L["Hin"]
            n_kg_load = L["n_kg"]
            is_head = li == 4
            n_mm_kg = 1 if is_head else n_kg_load
            n_chunks = (Hout + R - 1) // R

            # per-mg stats buffers [Mg, 64, 6]
            if not is_head:
                stat_t = [stats.tile([Mg, OWN, 6], F32, tag=f"sb{mg}",
                                     name=f"sb{li}_{mg}")
                          for mg in range(L["n_mg"])]

            for c in range(n_chunks):
                y0 = c * R
                rows = min(R, Hout - y0)
                used = (rows + 2) * WB  # data region size (from flat idx 1)
                tail = used + 1

                # ---- load input chunk per kgroup ----
                in_t = []
                for kg in range(n_kg_load):
                    it = inpool.tile([K if li == 0 else 96, FLAT], DT,
                                     tag=f"in{kg}")
                    nc.vector.memset(it[:, 0:1], 0.0)
                    nc.vector.memset(it[:, tail:tail + 1], 0.0)
                    if li == 0:
                        ch0 = kg * 128
                        nc.sync.dma_start(
                            out=it[:, 1:1 + used],
                            in_=x_ext[ch0:ch0 + 128, y0:y0 + rows + 2, :])
                    else:
                        ch0 = kg * 96
                        src = y_dram[li - 1]
                        if y0 == 0:
                            nc.vector.memset(it[:, 1:1 + WB], 0.0)
                            nc.sync.dma_start(
                                out=it[:, 1 + WB:1 + used],
                                in_=src[ch0:ch0 + 96, 0:rows + 1, :])
                            na, nb = 1 + WB, 1 + used
                        else:
                            nc.sync.dma_start(
                                out=it[:, 1:1 + used],
                                in_=src[ch0:ch0 + 96, y0 - 1:y0 + rows + 1, :])
                            na, nb = 1, 1 + used
                        # normalize + relu (BN of previous layer), in place
                        nc.scalar.activation(
                            out=it[:, na:nb], in_=it[:, na:nb],
                            func=mybir.ActivationFunctionType.Relu,
                            bias=st_cur[kg][:, 1:2], scale=st_cur[kg][:, 0:1])
                        # zero the W pad columns (post-normalize)
                        v3 = it[:, 1:1 + used].rearrange(
                            "p (r w) -> p r w", w=WB)
                        nc.vector.memset(v3[:, :, 0:1], 0.0)
                        nc.vector.memset(v3[:, :, WB - 1:WB], 0.0)
                    in_t.append(it)

                # ---- head: magnitude sqrt(re^2+im^2) ----
                if is_head:
                    mag = inpool.tile([96, FLAT], DT, tag="in2")
                    lim = tail + 1
                    nc.vector.tensor_mul(mag[:, 0:lim], in_t[0][:, 0:lim],
                                         in_t[0][:, 0:lim])
                    # square imag in place (it has no further readers)
                    nc.vector.tensor_mul(in_t[1][:, 0:lim], in_t[1][:, 0:lim],
                                         in_t[1][:, 0:lim])
                    nc.vector.tensor_add(mag[:, 0:lim], mag[:, 0:lim],
                                         in_t[1][:, 0:lim])
                    nc.scalar.activation(
                        out=mag[:, 0:lim], in_=mag[:, 0:lim],
                        func=mybir.ActivationFunctionType.Sqrt)
                    mm_in = [mag]
                else:
                    mm_in = in_t

                # ---- matmul tiles: 2 output rows per PSUM tile ----
                n_t = (rows + 1) // 2
                for mg in range(L["n_mg"]):
                    m0 = mg * Mg
                    stg = stpool.tile([Mg, R * WB], F32 if is_head else DT,
                                      tag="st")
                    for j in range(n_t):
                        r2 = min(2, rows - 2 * j)
                        N = r2 * WB
                        ps = pspool.tile([Mg, N], F32, tag="ps")
                        nmm = 9 * n_mm_kg
                        i_mm = 0
                        for kg in range(n_mm_kg):
                            for t in range(9):
                                dy, dx = t // 3 - 1, t % 3 - 1
                                off = 1 + (2 * j + 1 + dy) * WB + dx
                                nc.tensor.matmul(
                                    ps[:],
                                    w_t[li][:, t, kg, m0:m0 + Mg],
                                    mm_in[kg][:, off:off + N],
                                    start=(i_mm == 0), stop=(i_mm == nmm - 1))
                                i_mm += 1
                        if not is_head and y0 < OWN:
                            psv = ps[:].rearrange("p (r w) -> p r w", w=WB)
                            slot = y0 + 2 * j
                            for r in range(r2):
                                nc.vector.bn_stats(
                                    out=stat_t[mg][:, slot + r:slot + r + 1, :],
                                    in_=psv[:, r:r + 1, 1:1 + W])
                        dst = stg[:, 2 * j * WB:2 * j * WB + N]
                        if is_head:
                            nc.vector.tensor_scalar_add(
                                out=dst, in0=ps[:], scalar1=hb_t[:])
                        else:
                            nc.vector.tensor_copy(out=dst, in_=ps[:])
                    if is_head:
                        nc.scalar.activation(
                            out=stg[0:1, 0:rows * WB], in_=stg[0:1, 0:rows * WB],
                            func=mybir.ActivationFunctionType.Sigmoid)
                        sv = stg[:, 0:rows * WB].rearrange(
                            "p (r w) -> p r w", w=WB)
                        nc.sync.dma_start(
                            out=out_ext[:, y0:y0 + rows, :],
                            in_=sv[:, :, 1:1 + W])
                    else:
                        nc.sync.dma_start(
                            out=y_dram[li][m0:m0 + Mg, y0:y0 + rows, :],
                            in_=stg[:, 0:rows * WB])

            # ---- BN stats: aggregate, all-reduce, make scale/shift ----
            if not is_head:
                for mg in range(L["n_mg"]):
                    m0 = mg * Mg
                    mv = small.tile([Mg, 2], F32, tag="mv")
                    nc.vector.bn_aggr(out=mv[:], in_=stat_t[mg][:])
                    sums = small.tile([Mg, 2], F32, tag="sums")
                    nc.vector.tensor_scalar_mul(
                        out=sums[:, 0:1], in0=mv[:, 0:1], scalar1=CNT_LOCAL)
                    sq = small.tile([Mg, 1], F32, tag="sq")
                    nc.vector.tensor_mul(sq[:], mv[:, 0:1], mv[:, 0:1])
                    nc.vector.tensor_add(sq[:], sq[:], mv[:, 1:2])
                    nc.vector.tensor_scalar_mul(
                        out=sums[:, 1:2], in0=sq[:], scalar1=CNT_LOCAL)
                    nc.sync.dma_start(out=cc_in[li][m0:m0 + Mg, :],
                                      in_=sums[:])
                nc.gpsimd.collective_compute(
                    "AllReduce", mybir.AluOpType.add,
                    replica_groups=[list(range(N_CORES))],
                    ins=[cc_in[li][:].opt()], outs=[cc_out[li][:].opt()])
                # consumer kgroups of the next layer read 96-channel slices
                nL = LAYERS[li + 1]
                st_cur = []
                for kg in range(nL["n_kg"]):
                    k0 = kg * 96
                    sr = small.tile([96, 2], F32, tag="sr")
                    nc.sync.dma_start(out=sr[:], in_=cc_out[li][k0:k0 + 96, :])
                    gbt = small.tile([96, 2], F32, tag="gbt")
                    nc.sync.dma_start(out=gbt[:], in_=gb_ext[li][k0:k0 + 96, :])
                    mean = small.tile([96, 1], F32, tag="mean")
                    nc.vector.tensor_scalar_mul(
                        out=mean[:], in0=sr[:, 0:1], scalar1=1.0 / CNT_TOTAL)
                    var = small.tile([96, 1], F32, tag="var")
                    nc.vector.tensor_scalar_mul(
                        out=var[:], in0=sr[:, 1:2], scalar1=1.0 / CNT_TOTAL)
                    msq = small.tile([96, 1], F32, tag="msq")
                    nc.vector.tensor_mul(msq[:], mean[:], mean[:])
                    nc.vector.tensor_sub(var[:], var[:], msq[:])
                    std = small.tile([96, 1], F32, tag="std")
                    nc.scalar.activation(
                        out=std[:], in_=var[:],
                        func=mybir.ActivationFunctionType.Sqrt,
                        bias=eps_t[0:96, :])
                    rstd = small.tile([96, 1], F32, tag="rstd")
                    nc.vector.reciprocal(out=rstd[:], in_=std[:])
                    st = stp.tile([96, 2], F32, tag=f"stv{kg}")
                    nc.vector.tensor_mul(st[:, 0:1], rstd[:], gbt[:, 0:1])
                    tmp2 = small.tile([96, 1], F32, tag="tmp2")
                    nc.vector.tensor_mul(tmp2[:], mean[:], st[:, 0:1])
                    nc.vector.tensor_sub(st[:, 1:2], gbt[:, 1:2], tmp2[:])
                    st_cur.append(st)

    nc.compile()
    _nc_cache[mode] = nc
    return nc


def _prep_inputs(x, w1r, w1i, g1, b1, w2r, w2i, g2, b2,
                 w3r, w3i, g3, b3, w4r, w4i, g4, b4, wc, bc, wg, bg,
                 mode):
    """Host-side shard + pack. Returns in_maps list of 8 dicts."""
    npdt = _npdt(mode)
    x = np.asarray(x, np.float32)

    # stacked block weights [Mtot, Cin, 3, 3]
    W1 = np.concatenate([w1r, w1i], axis=0)
    def blk(wr, wi):
        top = np.concatenate([wr, -wi], axis=1)
        bot = np.concatenate([wi, wr], axis=1)
        return np.concatenate([top, bot], axis=0)
    W2, W3, W4 = blk(w2r, w2i), blk(w3r, w3i), blk(w4r, w4i)
    W5 = np.concatenate([wc, wg], axis=0)
    Ws = [W1, W2, W3, W4, W5]

    def pack_w(Wf, K, nkg, flip):
        # -> [K, 9, nkg, Mtot] with t = ky*3+kx, k-groups along Cin
        if flip:
            Wf = Wf[:, :, ::-1, :]
        Mtot, Cin = Wf.shape[0], Wf.shape[1]
        a = Wf.transpose(2, 3, 1, 0).reshape(9, Cin, Mtot)  # [t, cin, m]
        a = a.reshape(9, nkg, K, Mtot).transpose(2, 0, 1, 3)  # [K,9,g,M]
        return np.ascontiguousarray(a, dtype=npdt)

    gbs = []
    for g, b in ((g1, b1), (g2, b2), (g3, b3), (g4, b4)):
        gs = np.concatenate([g, g]).astype(np.float32)
        bs = np.concatenate([b, b]).astype(np.float32)
        gbs.append(np.ascontiguousarray(np.stack([gs, bs], axis=1)))
    hb = np.concatenate([bc, bg]).astype(np.float32).reshape(3, 1)

    in_maps = []
    for core in range(N_CORES):
        b_idx, h = core // 2, core % 2
        xi = x[b_idx]
        if h == 1:
            xi = xi[:, ::-1, :]
        # x_shard [256, 70, WB]: row 0 zero (local -1), rows 1..69 = local 0..68
        xs = np.zeros((256, 70, WB), np.float32)
        xs[:, 1:70, 1:1 + W] = xi[:, 0:69, :]
        m = {"x": xs.astype(npdt), "hb": hb}
        for li, L in enumerate(LAYERS):
            nkg = L["n_kg"] if li != 4 else 1
            m[f"w{li + 1}"] = pack_w(Ws[li], L["K"], nkg, flip=(h == 1))
        for li in range(4):
            m[f"gb{li + 1}"] = gbs[li]
        in_maps.append(m)
    return in_maps


_runner_cache = {}


def _get_runner(mode):
    """Build the SPMD jit executable once; returns run(in_maps) -> list of
    per-core output dicts. Mirrors bass2jax.run_bass_via_pjrt but caches the
    jitted callable so repeated kernel() calls don't re-trace/re-compile."""
    if mode in _runner_cache:
        return _runner_cache[mode]
    import jax
    from concourse import bass2jax
    from jax.experimental.shard_map import shard_map
    from jax.sharding import Mesh, PartitionSpec

    nc = build_program(mode)
    bass2jax.install_neuronx_cc_hook()

    partition_name = (nc.partition_id_tensor.name
                      if nc.partition_id_tensor else None)
    in_names, out_names, out_avals, zero_outs = [], [], [], []
    for alloc in nc.m.functions[0].allocations:
        if not isinstance(alloc, mybir.MemoryLocationSet):
            continue
        name = alloc.memorylocations[0].name
        if alloc.kind == "ExternalInput":
            if name != partition_name:
                in_names.append(name)
        elif alloc.kind == "ExternalOutput":
            shape = tuple(alloc.tensor_shape)
            dtype = mybir.dt.np(alloc.dtype)
            out_names.append(name)
            out_avals.append(jax.core.ShapedArray(shape, dtype))
            zero_outs.append(np.zeros(shape, dtype))
    n_params, n_outs = len(in_names), len(out_avals)
    all_names = list(in_names + out_names)
    if partition_name is not None:
        all_names.append(partition_name)
    all_names = tuple(all_names)
    donate = tuple(range(n_params, n_params + n_outs))

    def _body(*args):
        operands = list(args)
        if partition_name is not None:
            operands.append(bass2jax.partition_id_tensor())
        outs = bass2jax._bass_exec_p.bind(
            *operands,
            out_avals=tuple(out_avals),
            in_names=all_names,
            out_names=tuple(out_names),
            lowering_input_output_aliases=(),
            sim_require_finite=True,
            sim_require_nnan=True,
            nc=nc,
        )
        return tuple(outs)

    devices = jax.devices()[:N_CORES]
    mesh = Mesh(np.asarray(devices), ("core",))
    in_specs = (PartitionSpec("core"),) * (n_params + n_outs)
    out_specs = (PartitionSpec("core"),) * n_outs
    sharded = jax.jit(
        shard_map(_body, mesh=mesh, in_specs=in_specs, out_specs=out_specs,
                  check_rep=False),
        donate_argnums=donate, keep_unused=True)

    def run(in_maps):
        concat_in = [
            np.concatenate([np.asarray(in_maps[c][nm]) for c in
                            range(N_CORES)], axis=0)
            for nm in in_names
        ]
        concat_zeros = [
            np.zeros((N_CORES * z.shape[0], *z.shape[1:]), z.dtype)
            for z in zero_outs
        ]
        out_arrs = sharded(*concat_in, *concat_zeros)
        return [
            {nm: np.asarray(out_arrs[i]).reshape(N_CORES, *out_avals[i].shape)[c]
             for i, nm in enumerate(out_names)}
            for c in range(N_CORES)
        ]

    def time_device(in_maps, reps=5):
        """Time executions with inputs pre-staged on device (excludes host
        prep and host->device transfer). Returns list of seconds."""
        import time as _time
        from jax.sharding import NamedSharding
        concat_in = [
            np.concatenate([np.asarray(in_maps[c][nm]) for c in
                            range(N_CORES)], axis=0)
            for nm in in_names
        ]
        sh = NamedSharding(mesh, PartitionSpec("core"))
        dev_in = [jax.device_put(a, sh) for a in concat_in]
        for a in dev_in:
            a.block_until_ready()
        times = []
        for _ in range(reps):
            concat_zeros = [
                jax.device_put(
                    np.zeros((N_CORES * z.shape[0], *z.shape[1:]), z.dtype),
                    sh)
                for z in zero_outs
            ]
            for a in concat_zeros:
                a.block_until_ready()
            t0 = _time.time()
            out_arrs = sharded(*dev_in, *concat_zeros)
            for o in out_arrs:
                o.block_until_ready()
            times.append(_time.time() - t0)
        return times

    run.time_device = time_device
    _runner_cache[mode] = run
    return run


def kernel(**inputs):
    mode = DT_MODE
    run = _get_runner(mode)
    in_maps = _prep_inputs(mode=mode, **inputs)
    results = run(in_maps)
    out = np.zeros((4, 3, H, W), np.float32)
    for core in range(N_CORES):
        b_idx, h = core // 2, core % 2
        oc = results[core]["out"]  # [3, 64, W]
        if h == 0:
            out[b_idx, :, 0:OWN, :] = oc
        else:
            out[b_idx, :, OWN:H, :] = oc[:, ::-1, :]
    return out



# revision 7
# speedup vs baseline: 50.1175x; 50.1175x over previous
"""Trainium2 Bass kernel for nn_ComplexFFTRadNet (complex CNN, 4 cconv+BN+ReLU
layers, |.| magnitude, two 3x3 conv heads, sigmoid on cls channel).

Sharding: 8 NeuronCores = batch(4) x H-halves(2). Each core computes 64 output
rows of one image. Bottom halves are vertically FLIPPED on the host (input rows
and conv-kernel dy both reversed) so that every core runs the identical SPMD
program: global image edge at local top, 5 rows of neighbor halo at local
bottom. BatchNorm statistics (training-style, over N,H,W) are computed locally
per channel with bn_stats/bn_aggr over each core's owned 64 rows and combined
with a tiny AllReduce per layer.

Convolution = 9 shifted-window matmuls accumulated in PSUM; channels on the
partition (contraction) axis; activations stored row-major [C, H, W+2] with
zero pad columns so all 9 taps are contiguous-offset reads of one SBUF tile.
Real/imag are stacked along channels, so a complex conv is one real conv with
the block weight matrix [[wr, -wi], [wi, wr]].
"""
import os
import sys
import numpy as np
from contextlib import ExitStack

sys.path.insert(0, "/opt/trn_rl_repo")

from concourse import bass, bass_utils, tile, mybir, bacc  # noqa: E402

try:
    import ml_dtypes
    _BF16 = ml_dtypes.bfloat16
except Exception:  # pragma: no cover
    _BF16 = None

N_CORES = 8
H, W = 128, 224
WB = W + 2          # padded width
OWN = 64            # owned rows per core
R = 8               # output rows per chunk
CNT_LOCAL = float(OWN * W)          # elements per channel per core
CNT_TOTAL = float(4 * H * W)        # elements per channel globally
BN_EPS = 1e-5

# matmul/storage dtype: "f32" (exact) or "bf16" (4x faster PE, ~0.5% err)
DT_MODE = os.environ.get("KERNEL_DT", "bf16")

# layer table: (n_kg_load, kg_ch, K, n_kg_mm, n_mg, M_total, H_in_data, H_out)
#   L1: x[256] -> stacked 288 (yr144,yi144), kgroups 2x128, mgroups 3x96
#   L2: 288 -> 192, kgroups 3x96, mgroups 2x96
#   L3, L4: 192 -> 192, kgroups 2x96, mgroups 2x96
#   L5 head: mag[96] -> 3
LAYERS = [
    dict(K=128, n_kg=2, Cin=256, n_mg=3, Mg=96, Mtot=288, Hin=69, Hout=68),
    dict(K=96, n_kg=3, Cin=288, n_mg=2, Mg=96, Mtot=192, Hin=68, Hout=67),
    dict(K=96, n_kg=2, Cin=192, n_mg=2, Mg=96, Mtot=192, Hin=67, Hout=66),
    dict(K=96, n_kg=2, Cin=192, n_mg=2, Mg=96, Mtot=192, Hin=66, Hout=65),
    dict(K=96, n_kg=2, Cin=192, n_mg=1, Mg=3, Mtot=3, Hin=65, Hout=64),
]

_nc_cache = {}


def _dt(mode):
    return mybir.dt.bfloat16 if mode == "bf16" else mybir.dt.float32


def _npdt(mode):
    return _BF16 if mode == "bf16" else np.float32


def build_program(mode):
    if mode in _nc_cache:
        return _nc_cache[mode]
    DT = _dt(mode)
    F32 = mybir.dt.float32
    R = 16 if mode == "bf16" else 8  # chunk rows (SBUF-limited for f32)
    nc = bacc.Bacc("TRN2", target_bir_lowering=False, debug=False,
                   num_devices=N_CORES)

    # ---- external I/O ----
    x_ext = nc.dram_tensor("x", [256, 70, WB], DT, kind="ExternalInput").ap()
    w_ext = []
    for li, L in enumerate(LAYERS):
        w_ext.append(nc.dram_tensor(
            f"w{li + 1}", [L["K"], 9, L["n_kg"] if li != 4 else 1, L["Mtot"]],
            DT, kind="ExternalInput").ap())
    gb_ext = []
    for li in range(4):
        gb_ext.append(nc.dram_tensor(
            f"gb{li + 1}", [LAYERS[li]["Mtot"], 2], F32,
            kind="ExternalInput").ap())
    hb_ext = nc.dram_tensor("hb", [3, 1], F32, kind="ExternalInput").ap()
    out_ext = nc.dram_tensor("out", [3, OWN, W], F32,
                             kind="ExternalOutput").ap()

    FLAT = (R + 2) * WB + 2  # flat in-tile size (1 lead + rows+2 + 1 tail)

    with tile.TileContext(nc) as tc, ExitStack() as ctx:
        wpool = ctx.enter_context(tc.tile_pool(name="wts", bufs=2))
        inpool = ctx.enter_context(tc.tile_pool(name="inp", bufs=2))
        stpool = ctx.enter_context(tc.tile_pool(name="stage", bufs=3))
        pspool = ctx.enter_context(tc.tile_pool(name="ps", bufs=8, space="PSUM"))
        stats = ctx.enter_context(tc.tile_pool(name="stats", bufs=1))
        small = ctx.enter_context(tc.tile_pool(name="small", bufs=4))
        stp = ctx.enter_context(tc.tile_pool(name="stv", bufs=2))
        dram = ctx.enter_context(tc.tile_pool(name="dram", bufs=1, space="DRAM"))

        # constants
        eps_t = small.tile([128, 1], F32, tag="eps")
        nc.vector.memset(eps_t[:], BN_EPS)
        hb_t = small.tile([3, 1], F32, tag="hb")
        nc.sync.dma_start(out=hb_t[:], in_=hb_ext)

        # per-layer weights resident whole kernel; w1 first (startup path)
        w_t = []
        for li, L in enumerate(LAYERS):
            nkg = L["n_kg"] if li != 4 else 1
            t = wpool.tile([L["K"], 9, nkg, L["Mtot"]], DT, tag="w",
                           name=f"wt{li}")
            nc.sync.dma_start(out=t[:], in_=w_ext[li])
            w_t.append(t)

        # preload all BN gamma/beta slices (per consumer kgroup) upfront
        gb_t = []  # gb_t[li][kg] -> [96,2] f32
        for li in range(4):
            nkg_next = LAYERS[li + 1]["n_kg"]
            tiles = []
            for kg in range(nkg_next):
                k0 = kg * 96
                g = small.tile([96, 2], F32, tag=f"gb{li}_{kg}",
                               name=f"gbt{li}_{kg}")
                nc.sync.dma_start(out=g[:], in_=gb_ext[li][k0:k0 + 96, :])
                tiles.append(g)
            gb_t.append(tiles)

        # DRAM spill buffers for layer outputs (raw conv out, pre-BN)
        y_dram = []
        for li in range(4):
            L = LAYERS[li]
            y_dram.append(dram.tile([L["Mtot"], L["Hout"], WB], DT,
                                    tag=f"y{li}", name=f"y{li}"))
        cc_in = [dram.tile([LAYERS[li]["Mtot"], 2], F32, tag=f"cci{li}",
                           name=f"cci{li}")
                 for li in range(4)]
        cc_out = [dram.tile([LAYERS[li]["Mtot"], 2], F32, tag=f"cco{li}",
                            name=f"cco{li}")
                  for li in range(4)]

        st_cur = None  # list per kgroup of [96,2] tiles (s=col0, t=col1)

        def emit_boundary(li, L, stat_t):
            """BN aggregate + AllReduce + next-layer scale/shift.

            Emitted right after the last owned-row chunk so the collective
            overlaps the halo-row matmuls. Small arithmetic runs on the idle
            GpSimd queue; only the sqrt needs the Scalar engine."""
            Mg = L["Mg"]
            for mg in range(L["n_mg"]):
                m0 = mg * Mg
                mv = small.tile([Mg, 2], F32, tag="mv")
                nc.vector.bn_aggr(out=mv[:], in_=stat_t[mg][:])
                sums = small.tile([Mg, 2], F32, tag="sums")
                nc.vector.tensor_scalar_mul(
                    out=sums[:, 0:1], in0=mv[:, 0:1], scalar1=CNT_LOCAL)
                sq = small.tile([Mg, 1], F32, tag="sq")
                nc.vector.tensor_mul(sq[:], mv[:, 0:1], mv[:, 0:1])
                nc.vector.tensor_add(sq[:], sq[:], mv[:, 1:2])
                nc.vector.tensor_scalar_mul(
                    out=sums[:, 1:2], in0=sq[:], scalar1=CNT_LOCAL)
                nc.sync.dma_start(out=cc_in[li][m0:m0 + Mg, :], in_=sums[:])
            nc.gpsimd.collective_compute(
                "AllReduce", mybir.AluOpType.add,
                replica_groups=[list(range(N_CORES))],
                ins=[cc_in[li][:].opt()], outs=[cc_out[li][:].opt()])
            nL = LAYERS[li + 1]
            sts = []
            for kg in range(nL["n_kg"]):
                k0 = kg * 96
                sr = small.tile([96, 2], F32, tag="sr")
                nc.sync.dma_start(out=sr[:], in_=cc_out[li][k0:k0 + 96, :])
                gbt = gb_t[li][kg]
                mean = small.tile([96, 1], F32, tag="mean")
                nc.gpsimd.tensor_scalar_mul(
                    out=mean[:], in0=sr[:, 0:1], scalar1=1.0 / CNT_TOTAL)
                var = small.tile([96, 1], F32, tag="var")
                nc.gpsimd.tensor_scalar_mul(
                    out=var[:], in0=sr[:, 1:2], scalar1=1.0 / CNT_TOTAL)
                msq = small.tile([96, 1], F32, tag="msq")
                nc.gpsimd.tensor_mul(msq[:], mean[:], mean[:])
                nc.gpsimd.tensor_sub(var[:], var[:], msq[:])
                nc.gpsimd.tensor_scalar_add(
                    out=var[:], in0=var[:], scalar1=BN_EPS)
                std = small.tile([96, 1], F32, tag="std")
                nc.scalar.activation(
                    out=std[:], in_=var[:],
                    func=mybir.ActivationFunctionType.Sqrt)
                rstd = small.tile([96, 1], F32, tag="rstd")
                nc.vector.reciprocal(out=rstd[:], in_=std[:])
                st = stp.tile([96, 2], F32, tag=f"stv{kg}", name=f"st{li}_{kg}")
                nc.gpsimd.tensor_mul(st[:, 0:1], gbt[:, 0:1], rstd[:])
                tmp2 = small.tile([96, 1], F32, tag="tmp2")
                nc.gpsimd.tensor_mul(tmp2[:], mean[:], st[:, 0:1])
                nc.gpsimd.tensor_sub(st[:, 1:2], gbt[:, 1:2], tmp2[:])
                sts.append(st)
            return sts

        for li, L in enumerate(LAYERS):
            K, Mg, Mtot, Hout, Hin = L["K"], L["Mg"], L["Mtot"], L["Hout"], L["Hin"]
            n_kg_load = L["n_kg"]
            is_head = li == 4
            n_mm_kg = 1 if is_head else n_kg_load
            n_chunks = (Hout + R - 1) // R
            owned_chunks = OWN // R
            st_next = None

            # per-mg stats buffers [Mg, 64, 6]
            if not is_head:
                stat_t = [stats.tile([Mg, OWN, 6], F32, tag=f"sb{mg}",
                                     name=f"sb{li}_{mg}")
                          for mg in range(L["n_mg"])]

            for c in range(n_chunks):
                y0 = c * R
                rows = min(R, Hout - y0)
                used = (rows + 2) * WB  # data region size (from flat idx 1)
                tail = used + 1

                # ---- load input chunk per kgroup ----
                in_t = []
                for kg in range(n_kg_load):
                    it = inpool.tile([K if li == 0 else 96, FLAT], DT,
                                     tag=f"in{kg}")
                    nc.vector.memset(it[:, 0:1], 0.0)
                    nc.vector.memset(it[:, tail:tail + 1], 0.0)
                    if li == 0:
                        ch0 = kg * 128
                        nc.sync.dma_start(
                            out=it[:, 1:1 + used],
                            in_=x_ext[ch0:ch0 + 128, y0:y0 + rows + 2, :])
                    else:
                        ch0 = kg * 96
                        src = y_dram[li - 1]
                        if y0 == 0:
                            nc.vector.memset(it[:, 1:1 + WB], 0.0)
                            nc.sync.dma_start(
                                out=it[:, 1 + WB:1 + used],
                                in_=src[ch0:ch0 + 96, 0:rows + 1, :])
                            na, nb = 1 + WB, 1 + used
                        else:
                            nc.sync.dma_start(
                                out=it[:, 1:1 + used],
                                in_=src[ch0:ch0 + 96, y0 - 1:y0 + rows + 1, :])
                            na, nb = 1, 1 + used
                        # normalize + relu (BN of previous layer), in place
                        nc.scalar.activation(
                            out=it[:, na:nb], in_=it[:, na:nb],
                            func=mybir.ActivationFunctionType.Relu,
                            bias=st_cur[kg][:, 1:2], scale=st_cur[kg][:, 0:1])
                        # zero the W pad columns (post-normalize)
                        v3 = it[:, 1:1 + used].rearrange(
                            "p (r w) -> p r w", w=WB)
                        nc.vector.memset(v3[:, :, 0:1], 0.0)
                        nc.vector.memset(v3[:, :, WB - 1:WB], 0.0)
                    in_t.append(it)

                # ---- head: magnitude sqrt(re^2+im^2) ----
                if is_head:
                    mag = inpool.tile([96, FLAT], DT, tag="in2")
                    lim = tail + 1
                    nc.vector.tensor_mul(mag[:, 0:lim], in_t[0][:, 0:lim],
                                         in_t[0][:, 0:lim])
                    # square imag in place (it has no further readers)
                    nc.vector.tensor_mul(in_t[1][:, 0:lim], in_t[1][:, 0:lim],
                                         in_t[1][:, 0:lim])
                    nc.vector.tensor_add(mag[:, 0:lim], mag[:, 0:lim],
                                         in_t[1][:, 0:lim])
                    nc.scalar.activation(
                        out=mag[:, 0:lim], in_=mag[:, 0:lim],
                        func=mybir.ActivationFunctionType.Sqrt)
                    mm_in = [mag]
                else:
                    mm_in = in_t

                # ---- matmul tiles: 2 output rows per PSUM tile ----
                n_t = (rows + 1) // 2
                for mg in range(L["n_mg"]):
                    m0 = mg * Mg
                    stg = stpool.tile([Mg, R * WB], F32 if is_head else DT,
                                      tag="st")
                    for j in range(n_t):
                        r2 = min(2, rows - 2 * j)
                        N = r2 * WB
                        ps = pspool.tile([Mg, N], F32, tag="ps")
                        nmm = 9 * n_mm_kg
                        i_mm = 0
                        for kg in range(n_mm_kg):
                            for t in range(9):
                                dy, dx = t // 3 - 1, t % 3 - 1
                                off = 1 + (2 * j + 1 + dy) * WB + dx
                                nc.tensor.matmul(
                                    ps[:],
                                    w_t[li][:, t, kg, m0:m0 + Mg],
                                    mm_in[kg][:, off:off + N],
                                    start=(i_mm == 0), stop=(i_mm == nmm - 1))
                                i_mm += 1
                        dst = stg[:, 2 * j * WB:2 * j * WB + N]
                        if is_head:
                            nc.vector.tensor_scalar_add(
                                out=dst, in0=ps[:], scalar1=hb_t[:])
                        else:
                            nc.vector.tensor_copy(out=dst, in_=ps[:])
                            if y0 < OWN:
                                # stats on the bf16 values the next layer
                                # will actually read (2x DVE rate vs f32
                                # psum); walrus requires 6 elems/partition
                                # out, so one op per row
                                slot = y0 + 2 * j
                                dv = dst.rearrange("p (r w) -> p r w", w=WB)
                                for r in range(r2):
                                    nc.vector.bn_stats(
                                        out=stat_t[mg][:, slot + r:
                                                       slot + r + 1, :],
                                        in_=dv[:, r:r + 1, 1:1 + W])
                    if is_head:
                        nc.scalar.activation(
                            out=stg[0:1, 0:rows * WB], in_=stg[0:1, 0:rows * WB],
                            func=mybir.ActivationFunctionType.Sigmoid)
                        sv = stg[:, 0:rows * WB].rearrange(
                            "p (r w) -> p r w", w=WB)
                        nc.sync.dma_start(
                            out=out_ext[:, y0:y0 + rows, :],
                            in_=sv[:, :, 1:1 + W])
                    else:
                        nc.sync.dma_start(
                            out=y_dram[li][m0:m0 + Mg, y0:y0 + rows, :],
                            in_=stg[:, 0:rows * WB])

                # all owned rows done -> kick stats AllReduce; the halo
                # chunk(s) below overlap the collective latency
                if c == owned_chunks - 1 and not is_head:
                    st_next = emit_boundary(li, L, stat_t)

            if not is_head:
                st_cur = st_next

    nc.compile()
    _nc_cache[mode] = nc
    return nc


def _prep_inputs(x, w1r, w1i, g1, b1, w2r, w2i, g2, b2,
                 w3r, w3i, g3, b3, w4r, w4i, g4, b4, wc, bc, wg, bg,
                 mode):
    """Host-side shard + pack. Returns in_maps list of 8 dicts."""
    npdt = _npdt(mode)
    x = np.asarray(x, np.float32)

    # stacked block weights [Mtot, Cin, 3, 3]
    W1 = np.concatenate([w1r, w1i], axis=0)
    def blk(wr, wi):
        top = np.concatenate([wr, -wi], axis=1)
        bot = np.concatenate([wi, wr], axis=1)
        return np.concatenate([top, bot], axis=0)
    W2, W3, W4 = blk(w2r, w2i), blk(w3r, w3i), blk(w4r, w4i)
    W5 = np.concatenate([wc, wg], axis=0)
    Ws = [W1, W2, W3, W4, W5]

    def pack_w(Wf, K, nkg, flip):
        # -> [K, 9, nkg, Mtot] with t = ky*3+kx, k-groups along Cin
        if flip:
            Wf = Wf[:, :, ::-1, :]
        Mtot, Cin = Wf.shape[0], Wf.shape[1]
        a = Wf.transpose(2, 3, 1, 0).reshape(9, Cin, Mtot)  # [t, cin, m]
        a = a.reshape(9, nkg, K, Mtot).transpose(2, 0, 1, 3)  # [K,9,g,M]
        return np.ascontiguousarray(a, dtype=npdt)

    gbs = []
    for g, b in ((g1, b1), (g2, b2), (g3, b3), (g4, b4)):
        gs = np.concatenate([g, g]).astype(np.float32)
        bs = np.concatenate([b, b]).astype(np.float32)
        gbs.append(np.ascontiguousarray(np.stack([gs, bs], axis=1)))
    hb = np.concatenate([bc, bg]).astype(np.float32).reshape(3, 1)

    in_maps = []
    for core in range(N_CORES):
        b_idx, h = core // 2, core % 2
        xi = x[b_idx]
        if h == 1:
            xi = xi[:, ::-1, :]
        # x_shard [256, 70, WB]: row 0 zero (local -1), rows 1..69 = local 0..68
        xs = np.zeros((256, 70, WB), np.float32)
        xs[:, 1:70, 1:1 + W] = xi[:, 0:69, :]
        m = {"x": xs.astype(npdt), "hb": hb}
        for li, L in enumerate(LAYERS):
            nkg = L["n_kg"] if li != 4 else 1
            m[f"w{li + 1}"] = pack_w(Ws[li], L["K"], nkg, flip=(h == 1))
        for li in range(4):
            m[f"gb{li + 1}"] = gbs[li]
        in_maps.append(m)
    return in_maps


_runner_cache = {}


def _get_runner(mode):
    """Build the SPMD jit executable once; returns run(in_maps) -> list of
    per-core output dicts. Mirrors bass2jax.run_bass_via_pjrt but caches the
    jitted callable so repeated kernel() calls don't re-trace/re-compile."""
    if mode in _runner_cache:
        return _runner_cache[mode]
    import jax
    from concourse import bass2jax
    from jax.experimental.shard_map import shard_map
    from jax.sharding import Mesh, PartitionSpec

    nc = build_program(mode)
    bass2jax.install_neuronx_cc_hook()

    partition_name = (nc.partition_id_tensor.name
                      if nc.partition_id_tensor else None)
    in_names, out_names, out_avals, zero_outs = [], [], [], []
    for alloc in nc.m.functions[0].allocations:
        if not isinstance(alloc, mybir.MemoryLocationSet):
            continue
        name = alloc.memorylocations[0].name
        if alloc.kind == "ExternalInput":
            if name != partition_name:
                in_names.append(name)
        elif alloc.kind == "ExternalOutput":
            shape = tuple(alloc.tensor_shape)
            dtype = mybir.dt.np(alloc.dtype)
            out_names.append(name)
            out_avals.append(jax.core.ShapedArray(shape, dtype))
            zero_outs.append(np.zeros(shape, dtype))
    n_params, n_outs = len(in_names), len(out_avals)
    all_names = list(in_names + out_names)
    if partition_name is not None:
        all_names.append(partition_name)
    all_names = tuple(all_names)
    donate = tuple(range(n_params, n_params + n_outs))

    def _body(*args):
        operands = list(args)
        if partition_name is not None:
            operands.append(bass2jax.partition_id_tensor())
        outs = bass2jax._bass_exec_p.bind(
            *operands,
            out_avals=tuple(out_avals),
            in_names=all_names,
            out_names=tuple(out_names),
            lowering_input_output_aliases=(),
            sim_require_finite=True,
            sim_require_nnan=True,
            nc=nc,
        )
        return tuple(outs)

    devices = jax.devices()[:N_CORES]
    mesh = Mesh(np.asarray(devices), ("core",))
    in_specs = (PartitionSpec("core"),) * (n_params + n_outs)
    out_specs = (PartitionSpec("core"),) * n_outs
    sharded = jax.jit(
        shard_map(_body, mesh=mesh, in_specs=in_specs, out_specs=out_specs,
                  check_rep=False),
        donate_argnums=donate, keep_unused=True)

    def run(in_maps):
        concat_in = [
            np.concatenate([np.asarray(in_maps[c][nm]) for c in
                            range(N_CORES)], axis=0)
            for nm in in_names
        ]
        concat_zeros = [
            np.zeros((N_CORES * z.shape[0], *z.shape[1:]), z.dtype)
            for z in zero_outs
        ]
        out_arrs = sharded(*concat_in, *concat_zeros)
        return [
            {nm: np.asarray(out_arrs[i]).reshape(N_CORES, *out_avals[i].shape)[c]
             for i, nm in enumerate(out_names)}
            for c in range(N_CORES)
        ]

    def time_device(in_maps, reps=5):
        """Time executions with inputs pre-staged on device (excludes host
        prep and host->device transfer). Returns list of seconds."""
        import time as _time
        from jax.sharding import NamedSharding
        concat_in = [
            np.concatenate([np.asarray(in_maps[c][nm]) for c in
                            range(N_CORES)], axis=0)
            for nm in in_names
        ]
        sh = NamedSharding(mesh, PartitionSpec("core"))
        dev_in = [jax.device_put(a, sh) for a in concat_in]
        for a in dev_in:
            a.block_until_ready()
        times = []
        for _ in range(reps):
            concat_zeros = [
                jax.device_put(
                    np.zeros((N_CORES * z.shape[0], *z.shape[1:]), z.dtype),
                    sh)
                for z in zero_outs
            ]
            for a in concat_zeros:
                a.block_until_ready()
            t0 = _time.time()
            out_arrs = sharded(*dev_in, *concat_zeros)
            for o in out_arrs:
                o.block_until_ready()
            times.append(_time.time() - t0)
        return times

    run.time_device = time_device
    _runner_cache[mode] = run
    return run


def kernel(**inputs):
    mode = DT_MODE
    run = _get_runner(mode)
    in_maps = _prep_inputs(mode=mode, **inputs)
    results = run(in_maps)
    out = np.zeros((4, 3, H, W), np.float32)
    for core in range(N_CORES):
        b_idx, h = core // 2, core % 2
        oc = results[core]["out"]  # [3, 64, W]
        if h == 0:
            out[b_idx, :, 0:OWN, :] = oc
        else:
            out[b_idx, :, OWN:H, :] = oc[:, ::-1, :]
    return out



# revision 10
# speedup vs baseline: 50.8934x; 1.0155x over previous
"""Trainium2 Bass kernel for nn_ComplexFFTRadNet (complex CNN, 4 cconv+BN+ReLU
layers, |.| magnitude, two 3x3 conv heads, sigmoid on cls channel).

Sharding: 8 NeuronCores = batch(4) x H-halves(2). Each core computes 64 output
rows of one image. Bottom halves are vertically FLIPPED on the host (input rows
and conv-kernel dy both reversed) so that every core runs the identical SPMD
program: global image edge at local top, 5 rows of neighbor halo at local
bottom. BatchNorm statistics (training-style, over N,H,W) are computed locally
per channel with bn_stats/bn_aggr over each core's owned 64 rows and combined
with a tiny AllReduce per layer.

Convolution = 9 shifted-window matmuls accumulated in PSUM; channels on the
partition (contraction) axis; activations stored row-major [C, H, W+2] with
zero pad columns so all 9 taps are contiguous-offset reads of one SBUF tile.
Real/imag are stacked along channels, so a complex conv is one real conv with
the block weight matrix [[wr, -wi], [wi, wr]].
"""
import os
import sys
import numpy as np
from contextlib import ExitStack

sys.path.insert(0, "/opt/trn_rl_repo")

from concourse import bass, bass_utils, tile, mybir, bacc  # noqa: E402

try:
    import ml_dtypes
    _BF16 = ml_dtypes.bfloat16
except Exception:  # pragma: no cover
    _BF16 = None

N_CORES = 8
H, W = 128, 224
WB = W + 2          # padded width
OWN = 64            # owned rows per core
R = 8               # output rows per chunk
CNT_LOCAL = float(OWN * W)          # elements per channel per core
CNT_TOTAL = float(4 * H * W)        # elements per channel globally
BN_EPS = 1e-5

# matmul/storage dtype: "f32" (exact) or "bf16" (4x faster PE, ~0.5% err)
DT_MODE = os.environ.get("KERNEL_DT", "bf16")

# layer table: (n_kg_load, kg_ch, K, n_kg_mm, n_mg, M_total, H_in_data, H_out)
#   L1: x[256] -> stacked 288 (yr144,yi144), kgroups 2x128, mgroups 3x96
#   L2: 288 -> 192, kgroups 3x96, mgroups 2x96
#   L3, L4: 192 -> 192, kgroups 2x96, mgroups 2x96
#   L5 head: mag[96] -> 3
LAYERS = [
    dict(K=128, n_kg=2, Cin=256, n_mg=3, Mg=96, Mtot=288, Hin=69, Hout=68),
    dict(K=96, n_kg=3, Cin=288, n_mg=2, Mg=96, Mtot=192, Hin=68, Hout=67),
    dict(K=96, n_kg=2, Cin=192, n_mg=2, Mg=96, Mtot=192, Hin=67, Hout=66),
    dict(K=96, n_kg=2, Cin=192, n_mg=2, Mg=96, Mtot=192, Hin=66, Hout=65),
    dict(K=96, n_kg=2, Cin=192, n_mg=1, Mg=3, Mtot=3, Hin=65, Hout=64),
]

_nc_cache = {}


def _dt(mode):
    return mybir.dt.bfloat16 if mode == "bf16" else mybir.dt.float32


def _npdt(mode):
    return _BF16 if mode == "bf16" else np.float32


def build_program(mode):
    if mode in _nc_cache:
        return _nc_cache[mode]
    DT = _dt(mode)
    F32 = mybir.dt.float32
    R = 16 if mode == "bf16" else 8  # chunk rows (SBUF-limited for f32)
    nc = bacc.Bacc("TRN2", target_bir_lowering=False, debug=False,
                   num_devices=N_CORES)

    # ---- external I/O ----
    x_ext = nc.dram_tensor("x", [256, 70, WB], DT, kind="ExternalInput").ap()
    w_ext = []
    for li, L in enumerate(LAYERS):
        w_ext.append(nc.dram_tensor(
            f"w{li + 1}", [L["K"], 9, L["n_kg"] if li != 4 else 1, L["Mtot"]],
            DT, kind="ExternalInput").ap())
    gb_ext = []
    for li in range(4):
        gb_ext.append(nc.dram_tensor(
            f"gb{li + 1}", [LAYERS[li]["Mtot"], 2], F32,
            kind="ExternalInput").ap())
    hb_ext = nc.dram_tensor("hb", [3, 1], F32, kind="ExternalInput").ap()
    out_ext = nc.dram_tensor("out", [3, OWN, W], F32,
                             kind="ExternalOutput").ap()

    FLAT = (R + 2) * WB + 2  # flat in-tile size (1 lead + rows+2 + 1 tail)

    with tile.TileContext(nc) as tc, ExitStack() as ctx:
        wpool = ctx.enter_context(tc.tile_pool(name="wts", bufs=2))
        inpool = ctx.enter_context(tc.tile_pool(name="inp", bufs=2))
        stpool = ctx.enter_context(tc.tile_pool(name="stage", bufs=3))
        pspool = ctx.enter_context(tc.tile_pool(name="ps", bufs=8, space="PSUM"))
        stats = ctx.enter_context(tc.tile_pool(name="stats", bufs=1))
        small = ctx.enter_context(tc.tile_pool(name="small", bufs=4))
        stp = ctx.enter_context(tc.tile_pool(name="stv", bufs=2))
        dram = ctx.enter_context(tc.tile_pool(name="dram", bufs=1, space="DRAM"))

        # constants
        eps_t = small.tile([128, 1], F32, tag="eps")
        nc.vector.memset(eps_t[:], BN_EPS)
        hb_t = small.tile([3, 1], F32, tag="hb")
        nc.sync.dma_start(out=hb_t[:], in_=hb_ext)

        # per-layer weights resident whole kernel; w1 first (startup path)
        w_t = []
        for li, L in enumerate(LAYERS):
            nkg = L["n_kg"] if li != 4 else 1
            t = wpool.tile([L["K"], 9, nkg, L["Mtot"]], DT, tag="w",
                           name=f"wt{li}")
            nc.sync.dma_start(out=t[:], in_=w_ext[li])
            w_t.append(t)

        # preload all BN gamma/beta slices (per consumer kgroup) upfront
        gb_t = []  # gb_t[li][kg] -> [96,2] f32
        for li in range(4):
            nkg_next = LAYERS[li + 1]["n_kg"]
            tiles = []
            for kg in range(nkg_next):
                k0 = kg * 96
                g = small.tile([96, 2], F32, tag=f"gb{li}_{kg}",
                               name=f"gbt{li}_{kg}")
                nc.sync.dma_start(out=g[:], in_=gb_ext[li][k0:k0 + 96, :])
                tiles.append(g)
            gb_t.append(tiles)

        # DRAM spill buffers for layer outputs (raw conv out, pre-BN)
        y_dram = []
        for li in range(4):
            L = LAYERS[li]
            y_dram.append(dram.tile([L["Mtot"], L["Hout"], WB], DT,
                                    tag=f"y{li}", name=f"y{li}"))
        cc_in = [dram.tile([LAYERS[li]["Mtot"], 2], F32, tag=f"cci{li}",
                           name=f"cci{li}")
                 for li in range(4)]
        # Shared address space: the HBM-HBM AllReduce writes peers directly
        cc_out = [nc.dram_tensor(f"cco{li}", [LAYERS[li]["Mtot"], 2], F32,
                                 kind="Internal", addr_space="Shared").ap()
                  for li in range(4)]
        # tiny warm-up collective: absorbs cross-core NEFF-start skew during
        # L1 compute so the first real BN AllReduce doesn't pay it
        cc_win = dram.tile([1, 1], F32, tag="ccw", name="ccw")
        cc_wout = nc.dram_tensor("ccw_out", [1, 1], F32,
                                 kind="Internal", addr_space="Shared").ap()

        st_cur = None  # list per kgroup of [96,2] tiles (s=col0, t=col1)

        def emit_boundary(li, L, stat_t):
            """BN aggregate + AllReduce + next-layer scale/shift.

            Emitted right after the last owned-row chunk so the collective
            overlaps the halo-row matmuls. Small arithmetic runs on the idle
            GpSimd queue; only the sqrt needs the Scalar engine."""
            Mg = L["Mg"]
            for mg in range(L["n_mg"]):
                m0 = mg * Mg
                mv = small.tile([Mg, 2], F32, tag="mv")
                nc.vector.bn_aggr(out=mv[:], in_=stat_t[mg][:])
                sums = small.tile([Mg, 2], F32, tag="sums")
                nc.vector.tensor_scalar_mul(
                    out=sums[:, 0:1], in0=mv[:, 0:1], scalar1=CNT_LOCAL)
                sq = small.tile([Mg, 1], F32, tag="sq")
                nc.vector.tensor_mul(sq[:], mv[:, 0:1], mv[:, 0:1])
                nc.vector.tensor_add(sq[:], sq[:], mv[:, 1:2])
                nc.vector.tensor_scalar_mul(
                    out=sums[:, 1:2], in0=sq[:], scalar1=CNT_LOCAL)
                nc.sync.dma_start(out=cc_in[li][m0:m0 + Mg, :], in_=sums[:])
            nc.gpsimd.collective_compute(
                "AllReduce", mybir.AluOpType.add,
                replica_groups=[list(range(N_CORES))],
                ins=[cc_in[li][:].opt()], outs=[cc_out[li][:].opt()])
            nL = LAYERS[li + 1]
            sts = []
            for kg in range(nL["n_kg"]):
                k0 = kg * 96
                sr = small.tile([96, 2], F32, tag="sr")
                nc.sync.dma_start(out=sr[:], in_=cc_out[li][k0:k0 + 96, :])
                gbt = gb_t[li][kg]
                mean = small.tile([96, 1], F32, tag="mean")
                nc.gpsimd.tensor_scalar_mul(
                    out=mean[:], in0=sr[:, 0:1], scalar1=1.0 / CNT_TOTAL)
                var = small.tile([96, 1], F32, tag="var")
                nc.gpsimd.tensor_scalar_mul(
                    out=var[:], in0=sr[:, 1:2], scalar1=1.0 / CNT_TOTAL)
                msq = small.tile([96, 1], F32, tag="msq")
                nc.gpsimd.tensor_mul(msq[:], mean[:], mean[:])
                nc.gpsimd.tensor_sub(var[:], var[:], msq[:])
                nc.gpsimd.tensor_scalar_add(
                    out=var[:], in0=var[:], scalar1=BN_EPS)
                std = small.tile([96, 1], F32, tag="std")
                nc.scalar.activation(
                    out=std[:], in_=var[:],
                    func=mybir.ActivationFunctionType.Sqrt)
                rstd = small.tile([96, 1], F32, tag="rstd")
                nc.vector.reciprocal(out=rstd[:], in_=std[:])
                st = stp.tile([96, 2], F32, tag=f"stv{kg}", name=f"st{li}_{kg}")
                nc.gpsimd.tensor_mul(st[:, 0:1], gbt[:, 0:1], rstd[:])
                tmp2 = small.tile([96, 1], F32, tag="tmp2")
                nc.gpsimd.tensor_mul(tmp2[:], mean[:], st[:, 0:1])
                nc.gpsimd.tensor_sub(st[:, 1:2], gbt[:, 1:2], tmp2[:])
                sts.append(st)
            return sts

        for li, L in enumerate(LAYERS):
            K, Mg, Mtot, Hout, Hin = L["K"], L["Mg"], L["Mtot"], L["Hout"], L["Hin"]
            n_kg_load = L["n_kg"]
            is_head = li == 4
            n_mm_kg = 1 if is_head else n_kg_load
            n_chunks = (Hout + R - 1) // R
            owned_chunks = OWN // R
            st_next = None

            # per-mg stats buffers [Mg, 64, 6]
            if not is_head:
                stat_t = [stats.tile([Mg, OWN, 6], F32, tag=f"sb{mg}",
                                     name=f"sb{li}_{mg}")
                          for mg in range(L["n_mg"])]

            for c in range(n_chunks):
                y0 = c * R
                rows = min(R, Hout - y0)
                used = (rows + 2) * WB  # data region size (from flat idx 1)
                tail = used + 1

                # ---- load input chunk per kgroup ----
                in_t = []
                for kg in range(n_kg_load):
                    it = inpool.tile([K if li == 0 else 96, FLAT], DT,
                                     tag=f"in{kg}")
                    nc.vector.memset(it[:, 0:1], 0.0)
                    nc.vector.memset(it[:, tail:tail + 1], 0.0)
                    if li == 0:
                        ch0 = kg * 128
                        nc.sync.dma_start(
                            out=it[:, 1:1 + used],
                            in_=x_ext[ch0:ch0 + 128, y0:y0 + rows + 2, :])
                    else:
                        ch0 = kg * 96
                        src = y_dram[li - 1]
                        if y0 == 0:
                            nc.vector.memset(it[:, 1:1 + WB], 0.0)
                            nc.sync.dma_start(
                                out=it[:, 1 + WB:1 + used],
                                in_=src[ch0:ch0 + 96, 0:rows + 1, :])
                            na, nb = 1 + WB, 1 + used
                        else:
                            nc.sync.dma_start(
                                out=it[:, 1:1 + used],
                                in_=src[ch0:ch0 + 96, y0 - 1:y0 + rows + 1, :])
                            na, nb = 1, 1 + used
                        # normalize + relu (BN of previous layer), in place
                        nc.scalar.activation(
                            out=it[:, na:nb], in_=it[:, na:nb],
                            func=mybir.ActivationFunctionType.Relu,
                            bias=st_cur[kg][:, 1:2], scale=st_cur[kg][:, 0:1])
                        # zero the W pad columns (post-normalize)
                        v3 = it[:, 1:1 + used].rearrange(
                            "p (r w) -> p r w", w=WB)
                        nc.vector.memset(v3[:, :, 0:1], 0.0)
                        nc.vector.memset(v3[:, :, WB - 1:WB], 0.0)
                    in_t.append(it)

                # ---- head: magnitude sqrt(re^2+im^2) ----
                if is_head:
                    mag = inpool.tile([96, FLAT], DT, tag="in2")
                    lim = tail + 1
                    nc.vector.tensor_mul(mag[:, 0:lim], in_t[0][:, 0:lim],
                                         in_t[0][:, 0:lim])
                    # square imag in place (it has no further readers)
                    nc.vector.tensor_mul(in_t[1][:, 0:lim], in_t[1][:, 0:lim],
                                         in_t[1][:, 0:lim])
                    nc.vector.tensor_add(mag[:, 0:lim], mag[:, 0:lim],
                                         in_t[1][:, 0:lim])
                    nc.scalar.activation(
                        out=mag[:, 0:lim], in_=mag[:, 0:lim],
                        func=mybir.ActivationFunctionType.Sqrt)
                    mm_in = [mag]
                else:
                    mm_in = in_t

                # ---- matmul tiles: 2 output rows per PSUM tile ----
                n_t = (rows + 1) // 2
                for mg in range(L["n_mg"]):
                    m0 = mg * Mg
                    stg = stpool.tile([Mg, R * WB], F32 if is_head else DT,
                                      tag="st")
                    for j in range(n_t):
                        r2 = min(2, rows - 2 * j)
                        N = r2 * WB
                        ps = pspool.tile([Mg, N], F32, tag="ps")
                        nmm = 9 * n_mm_kg
                        i_mm = 0
                        for kg in range(n_mm_kg):
                            for t in range(9):
                                dy, dx = t // 3 - 1, t % 3 - 1
                                off = 1 + (2 * j + 1 + dy) * WB + dx
                                nc.tensor.matmul(
                                    ps[:],
                                    w_t[li][:, t, kg, m0:m0 + Mg],
                                    mm_in[kg][:, off:off + N],
                                    start=(i_mm == 0), stop=(i_mm == nmm - 1))
                                i_mm += 1
                        dst = stg[:, 2 * j * WB:2 * j * WB + N]
                        if is_head:
                            nc.vector.tensor_scalar_add(
                                out=dst, in0=ps[:], scalar1=hb_t[:])
                        else:
                            nc.vector.tensor_copy(out=dst, in_=ps[:])
                            if y0 < OWN:
                                # stats on the bf16 values the next layer
                                # will actually read (2x DVE rate vs f32
                                # psum); walrus requires 6 elems/partition
                                # out, so one op per row
                                slot = y0 + 2 * j
                                dv = dst.rearrange("p (r w) -> p r w", w=WB)
                                for r in range(r2):
                                    nc.vector.bn_stats(
                                        out=stat_t[mg][:, slot + r:
                                                       slot + r + 1, :],
                                        in_=dv[:, r:r + 1, 1:1 + W])
                    if is_head:
                        nc.scalar.activation(
                            out=stg[0:1, 0:rows * WB], in_=stg[0:1, 0:rows * WB],
                            func=mybir.ActivationFunctionType.Sigmoid)
                        sv = stg[:, 0:rows * WB].rearrange(
                            "p (r w) -> p r w", w=WB)
                        nc.sync.dma_start(
                            out=out_ext[:, y0:y0 + rows, :],
                            in_=sv[:, :, 1:1 + W])
                    else:
                        nc.sync.dma_start(
                            out=y_dram[li][m0:m0 + Mg, y0:y0 + rows, :],
                            in_=stg[:, 0:rows * WB])

                if li == 0 and c == 0:
                    nc.sync.dma_start(out=cc_win[:], in_=eps_t[0:1, :])
                    nc.gpsimd.collective_compute(
                        "AllReduce", mybir.AluOpType.add,
                        replica_groups=[list(range(N_CORES))],
                        ins=[cc_win[:].opt()], outs=[cc_wout[:].opt()])

                # all owned rows done -> kick stats AllReduce; the halo
                # chunk(s) below overlap the collective latency
                if c == owned_chunks - 1 and not is_head:
                    st_next = emit_boundary(li, L, stat_t)

            if not is_head:
                st_cur = st_next

    nc.compile()
    _nc_cache[mode] = nc
    return nc


def _prep_inputs(x, w1r, w1i, g1, b1, w2r, w2i, g2, b2,
                 w3r, w3i, g3, b3, w4r, w4i, g4, b4, wc, bc, wg, bg,
                 mode):
    """Host-side shard + pack. Returns in_maps list of 8 dicts."""
    npdt = _npdt(mode)
    x = np.asarray(x, np.float32)

    # stacked block weights [Mtot, Cin, 3, 3]
    W1 = np.concatenate([w1r, w1i], axis=0)
    def blk(wr, wi):
        top = np.concatenate([wr, -wi], axis=1)
        bot = np.concatenate([wi, wr], axis=1)
        return np.concatenate([top, bot], axis=0)
    W2, W3, W4 = blk(w2r, w2i), blk(w3r, w3i), blk(w4r, w4i)
    W5 = np.concatenate([wc, wg], axis=0)
    Ws = [W1, W2, W3, W4, W5]

    def pack_w(Wf, K, nkg, flip):
        # -> [K, 9, nkg, Mtot] with t = ky*3+kx, k-groups along Cin
        if flip:
            Wf = Wf[:, :, ::-1, :]
        Mtot, Cin = Wf.shape[0], Wf.shape[1]
        a = Wf.transpose(2, 3, 1, 0).reshape(9, Cin, Mtot)  # [t, cin, m]
        a = a.reshape(9, nkg, K, Mtot).transpose(2, 0, 1, 3)  # [K,9,g,M]
        return np.ascontiguousarray(a, dtype=npdt)

    gbs = []
    for g, b in ((g1, b1), (g2, b2), (g3, b3), (g4, b4)):
        gs = np.concatenate([g, g]).astype(np.float32)
        bs = np.concatenate([b, b]).astype(np.float32)
        gbs.append(np.ascontiguousarray(np.stack([gs, bs], axis=1)))
    hb = np.concatenate([bc, bg]).astype(np.float32).reshape(3, 1)

    in_maps = []
    for core in range(N_CORES):
        b_idx, h = core // 2, core % 2
        xi = x[b_idx]
        if h == 1:
            xi = xi[:, ::-1, :]
        # x_shard [256, 70, WB]: row 0 zero (local -1), rows 1..69 = local 0..68
        xs = np.zeros((256, 70, WB), np.float32)
        xs[:, 1:70, 1:1 + W] = xi[:, 0:69, :]
        m = {"x": xs.astype(npdt), "hb": hb}
        for li, L in enumerate(LAYERS):
            nkg = L["n_kg"] if li != 4 else 1
            m[f"w{li + 1}"] = pack_w(Ws[li], L["K"], nkg, flip=(h == 1))
        for li in range(4):
            m[f"gb{li + 1}"] = gbs[li]
        in_maps.append(m)
    return in_maps


_runner_cache = {}


def _get_runner(mode):
    """Build the SPMD jit executable once; returns run(in_maps) -> list of
    per-core output dicts. Mirrors bass2jax.run_bass_via_pjrt but caches the
    jitted callable so repeated kernel() calls don't re-trace/re-compile."""
    if mode in _runner_cache:
        return _runner_cache[mode]
    import jax
    from concourse import bass2jax
    from jax.experimental.shard_map import shard_map
    from jax.sharding import Mesh, PartitionSpec

    nc = build_program(mode)
    bass2jax.install_neuronx_cc_hook()

    partition_name = (nc.partition_id_tensor.name
                      if nc.partition_id_tensor else None)
    in_names, out_names, out_avals, zero_outs = [], [], [], []
    for alloc in nc.m.functions[0].allocations:
        if not isinstance(alloc, mybir.MemoryLocationSet):
            continue
        name = alloc.memorylocations[0].name
        if alloc.kind == "ExternalInput":
            if name != partition_name:
                in_names.append(name)
        elif alloc.kind == "ExternalOutput":
            shape = tuple(alloc.tensor_shape)
            dtype = mybir.dt.np(alloc.dtype)
            out_names.append(name)
            out_avals.append(jax.core.ShapedArray(shape, dtype))
            zero_outs.append(np.zeros(shape, dtype))
    n_params, n_outs = len(in_names), len(out_avals)
    all_names = list(in_names + out_names)
    if partition_name is not None:
        all_names.append(partition_name)
    all_names = tuple(all_names)
    donate = tuple(range(n_params, n_params + n_outs))

    def _body(*args):
        operands = list(args)
        if partition_name is not None:
            operands.append(bass2jax.partition_id_tensor())
        outs = bass2jax._bass_exec_p.bind(
            *operands,
            out_avals=tuple(out_avals),
            in_names=all_names,
            out_names=tuple(out_names),
            lowering_input_output_aliases=(),
            sim_require_finite=True,
            sim_require_nnan=True,
            nc=nc,
        )
        return tuple(outs)

    devices = jax.devices()[:N_CORES]
    mesh = Mesh(np.asarray(devices), ("core",))
    in_specs = (PartitionSpec("core"),) * (n_params + n_outs)
    out_specs = (PartitionSpec("core"),) * n_outs
    sharded = jax.jit(
        shard_map(_body, mesh=mesh, in_specs=in_specs, out_specs=out_specs,
                  check_rep=False),
        donate_argnums=donate, keep_unused=True)

    def run(in_maps):
        concat_in = [
            np.concatenate([np.asarray(in_maps[c][nm]) for c in
                            range(N_CORES)], axis=0)
            for nm in in_names
        ]
        concat_zeros = [
            np.zeros((N_CORES * z.shape[0], *z.shape[1:]), z.dtype)
            for z in zero_outs
        ]
        out_arrs = sharded(*concat_in, *concat_zeros)
        return [
            {nm: np.asarray(out_arrs[i]).reshape(N_CORES, *out_avals[i].shape)[c]
             for i, nm in enumerate(out_names)}
            for c in range(N_CORES)
        ]

    def time_device(in_maps, reps=5):
        """Time executions with inputs pre-staged on device (excludes host
        prep and host->device transfer). Returns list of seconds."""
        import time as _time
        from jax.sharding import NamedSharding
        concat_in = [
            np.concatenate([np.asarray(in_maps[c][nm]) for c in
                            range(N_CORES)], axis=0)
            for nm in in_names
        ]
        sh = NamedSharding(mesh, PartitionSpec("core"))
        dev_in = [jax.device_put(a, sh) for a in concat_in]
        for a in dev_in:
            a.block_until_ready()
        times = []
        for _ in range(reps):
            concat_zeros = [
                jax.device_put(
                    np.zeros((N_CORES * z.shape[0], *z.shape[1:]), z.dtype),
                    sh)
                for z in zero_outs
            ]
            for a in concat_zeros:
                a.block_until_ready()
            t0 = _time.time()
            out_arrs = sharded(*dev_in, *concat_zeros)
            for o in out_arrs:
                o.block_until_ready()
            times.append(_time.time() - t0)
        return times

    run.time_device = time_device
    _runner_cache[mode] = run
    return run


def kernel(**inputs):
    mode = DT_MODE
    run = _get_runner(mode)
    in_maps = _prep_inputs(mode=mode, **inputs)
    results = run(in_maps)
    out = np.zeros((4, 3, H, W), np.float32)
    for core in range(N_CORES):
        b_idx, h = core // 2, core % 2
        oc = results[core]["out"]  # [3, 64, W]
        if h == 0:
            out[b_idx, :, 0:OWN, :] = oc
        else:
            out[b_idx, :, OWN:H, :] = oc[:, ::-1, :]
    return out



# revision 19
# speedup vs baseline: 51.2240x; 1.0065x over previous
"""Trainium2 Bass kernel for nn_ComplexFFTRadNet (complex CNN, 4 cconv+BN+ReLU
layers, |.| magnitude, two 3x3 conv heads, sigmoid on cls channel).

Sharding: 8 NeuronCores = batch(4) x H-halves(2). Each core computes 64 output
rows of one image. Bottom halves are vertically FLIPPED on the host (input rows
and conv-kernel dy both reversed) so that every core runs the identical SPMD
program: global image edge at local top, 5 rows of neighbor halo at local
bottom. BatchNorm statistics (training-style, over N,H,W) are computed locally
per channel with bn_stats/bn_aggr over each core's owned 64 rows and combined
with a tiny AllReduce per layer.

Convolution = 9 shifted-window matmuls accumulated in PSUM; channels on the
partition (contraction) axis; activations stored row-major [C, H, W+2] with
zero pad columns so all 9 taps are contiguous-offset reads of one SBUF tile.
Real/imag are stacked along channels, so a complex conv is one real conv with
the block weight matrix [[wr, -wi], [wi, wr]].
"""
import os
import sys
import numpy as np
from contextlib import ExitStack

sys.path.insert(0, "/opt/trn_rl_repo")

from concourse import bass, bass_utils, tile, mybir, bacc  # noqa: E402

try:
    import ml_dtypes
    _BF16 = ml_dtypes.bfloat16
except Exception:  # pragma: no cover
    _BF16 = None

N_CORES = 8
H, W = 128, 224
WB = W + 2          # padded width
OWN = 64            # owned rows per core
R = 8               # output rows per chunk
CNT_LOCAL = float(OWN * W)          # elements per channel per core
CNT_TOTAL = float(4 * H * W)        # elements per channel globally
BN_EPS = 1e-5

# matmul/storage dtype: "f32" (exact) or "bf16" (4x faster PE, ~0.5% err)
DT_MODE = os.environ.get("KERNEL_DT", "bf16")

# layer table: (n_kg_load, kg_ch, K, n_kg_mm, n_mg, M_total, H_in_data, H_out)
#   L1: x[256] -> stacked 288 (yr144,yi144), kgroups 2x128, mgroups 3x96
#   L2-L4: v3 mixed (ky,c) K-packing: contraction space 3*Cin flattened as
#     f = ky*Cin + c into full 128-partition groups; only the 3 kx taps
#     stream, cutting matmul pairs per 2-row tile (L2 27->21, L3/L4 18->15)
#   L5 head: mag[96] -> 3
LAYERS = [
    dict(K=128, n_kg=2, Cin=256, n_mg=3, Mg=96, Mtot=288, Hin=69, Hout=68),
    dict(K=96, n_kg=3, Cin=288, n_mg=2, Mg=96, Mtot=192, Hin=68, Hout=67,
         v3=True),
    dict(K=96, n_kg=2, Cin=192, n_mg=2, Mg=96, Mtot=192, Hin=67, Hout=66,
         v3=True),
    dict(K=96, n_kg=2, Cin=192, n_mg=2, Mg=96, Mtot=192, Hin=66, Hout=65,
         v3=True),
    dict(K=96, n_kg=2, Cin=192, n_mg=1, Mg=3, Mtot=3, Hin=65, Hout=64),
]
for _L in LAYERS:
    if _L.get("v3"):
        _KT = 3 * _L["Cin"]
        _L["G"] = (_KT + 127) // 128
        _L["rem"] = _KT - (_L["G"] - 1) * 128


def _dyruns(Cin, g, G, rem):
    """Partition runs of group g in the flattened (ky, c) space.

    Returns (p0, p1, ky, c0): partitions [p0,p1) of the group hold channels
    c0.. with row-tap ky (0/1/2 = input row offset ky-1)."""
    f0 = g * 128
    f1 = f0 + (128 if g < G - 1 else rem)
    runs = []
    f = f0
    while f < f1:
        ky, c0 = f // Cin, f % Cin
        fend = min(f1, (ky + 1) * Cin)
        runs.append((f - f0, fend - f0, ky, c0))
        f = fend
    return runs

_nc_cache = {}


def _dt(mode):
    return mybir.dt.bfloat16 if mode == "bf16" else mybir.dt.float32


def _npdt(mode):
    return _BF16 if mode == "bf16" else np.float32


def build_program(mode):
    if mode in _nc_cache:
        return _nc_cache[mode]
    DT = _dt(mode)
    F32 = mybir.dt.float32
    R = 16 if mode == "bf16" else 8  # chunk rows (SBUF-limited for f32)
    nc = bacc.Bacc("TRN2", target_bir_lowering=False, debug=False,
                   num_devices=N_CORES)

    # ---- external I/O ----
    x_ext = nc.dram_tensor("x", [256, 70, WB], DT, kind="ExternalInput").ap()
    w_ext = []
    for li, L in enumerate(LAYERS):
        if L.get("v3"):
            shape = [128, 3, L["G"], L["Mtot"]]
        else:
            shape = [L["K"], 9, L["n_kg"] if li != 4 else 1, L["Mtot"]]
        w_ext.append(nc.dram_tensor(
            f"w{li + 1}", shape, DT, kind="ExternalInput").ap())
    gb_ext = []
    for li in range(4):
        gb_ext.append(nc.dram_tensor(
            f"gb{li + 1}", [LAYERS[li]["Mtot"], 2], F32,
            kind="ExternalInput").ap())
    hb_ext = nc.dram_tensor("hb", [3, 1], F32, kind="ExternalInput").ap()
    out_ext = nc.dram_tensor("out", [3, OWN, W], F32,
                             kind="ExternalOutput").ap()

    FLAT = (R + 2) * WB + 2  # flat in-tile size (1 lead + rows+2 + 1 tail)

    with tile.TileContext(nc) as tc, ExitStack() as ctx:
        wpool = ctx.enter_context(tc.tile_pool(name="wts", bufs=2))
        inpool = ctx.enter_context(tc.tile_pool(name="inp", bufs=2))
        stpool = ctx.enter_context(tc.tile_pool(name="stage", bufs=3))
        pspool = ctx.enter_context(tc.tile_pool(name="ps", bufs=8, space="PSUM"))
        stats = ctx.enter_context(tc.tile_pool(name="stats", bufs=1))
        small = ctx.enter_context(tc.tile_pool(name="small", bufs=4))
        stp = ctx.enter_context(tc.tile_pool(name="stv", bufs=2))
        dram = ctx.enter_context(tc.tile_pool(name="dram", bufs=1, space="DRAM"))

        # constants
        eps_t = small.tile([128, 1], F32, tag="eps")
        nc.vector.memset(eps_t[:], BN_EPS)
        hb_t = small.tile([3, 1], F32, tag="hb")
        nc.sync.dma_start(out=hb_t[:], in_=hb_ext)

        # per-layer weights resident whole kernel; w1 first (startup path)
        w_t = []
        for li, L in enumerate(LAYERS):
            if L.get("v3"):
                t = wpool.tile([128, 3, L["G"], L["Mtot"]], DT, tag="w",
                               name=f"wt{li}")
            else:
                nkg = L["n_kg"] if li != 4 else 1
                t = wpool.tile([L["K"], 9, nkg, L["Mtot"]], DT, tag="w",
                               name=f"wt{li}")
            nc.sync.dma_start(out=t[:], in_=w_ext[li])
            w_t.append(t)

        # preload all BN gamma/beta slices (per consumer kgroup) upfront
        gb_t = []  # gb_t[li][kg] -> [96,2] f32
        for li in range(4):
            nkg_next = LAYERS[li + 1]["n_kg"]
            tiles = []
            for kg in range(nkg_next):
                k0 = kg * 96
                g = small.tile([96, 2], F32, tag=f"gb{li}_{kg}",
                               name=f"gbt{li}_{kg}")
                nc.sync.dma_start(out=g[:], in_=gb_ext[li][k0:k0 + 96, :])
                tiles.append(g)
            gb_t.append(tiles)

        # DRAM spill buffers for layer outputs (raw conv out, pre-BN)
        y_dram = []
        for li in range(4):
            L = LAYERS[li]
            y_dram.append(dram.tile([L["Mtot"], L["Hout"], WB], DT,
                                    tag=f"y{li}", name=f"y{li}"))
        cc_in = [dram.tile([LAYERS[li]["Mtot"], 2], F32, tag=f"cci{li}",
                           name=f"cci{li}")
                 for li in range(4)]
        # Shared address space: the HBM-HBM AllReduce writes peers directly
        cc_out = [nc.dram_tensor(f"cco{li}", [LAYERS[li]["Mtot"], 2], F32,
                                 kind="Internal", addr_space="Shared").ap()
                  for li in range(4)]
        # tiny warm-up collective: absorbs cross-core NEFF-start skew during
        # L1 compute so the first real BN AllReduce doesn't pay it
        cc_win = dram.tile([1, 1], F32, tag="ccw", name="ccw")
        cc_wout = nc.dram_tensor("ccw_out", [1, 1], F32,
                                 kind="Internal", addr_space="Shared").ap()
        # per-channel scale/shift staged via DRAM so v3 consumers can gather
        # them into the mixed (ky,c) partition layout
        st_dram = [dram.tile([LAYERS[li + 1]["Cin"], 2], F32, tag=f"std{li}",
                             name=f"std{li}")
                   for li in range(3)]

        st_cur = None  # list per kgroup of [96,2] tiles (s=col0, t=col1)

        def emit_boundary(li, L, stat_t):
            """BN aggregate + AllReduce + next-layer scale/shift.

            Emitted right after the last owned-row chunk so the collective
            overlaps the halo-row matmuls. Small arithmetic runs on the idle
            GpSimd queue; only the sqrt needs the Scalar engine."""
            Mg = L["Mg"]
            for mg in range(L["n_mg"]):
                m0 = mg * Mg
                mv = small.tile([Mg, 2], F32, tag="mv")
                nc.vector.bn_aggr(out=mv[:], in_=stat_t[mg][:])
                sums = small.tile([Mg, 2], F32, tag="sums")
                nc.vector.tensor_scalar_mul(
                    out=sums[:, 0:1], in0=mv[:, 0:1], scalar1=CNT_LOCAL)
                sq = small.tile([Mg, 1], F32, tag="sq")
                nc.vector.tensor_mul(sq[:], mv[:, 0:1], mv[:, 0:1])
                nc.vector.tensor_add(sq[:], sq[:], mv[:, 1:2])
                nc.vector.tensor_scalar_mul(
                    out=sums[:, 1:2], in0=sq[:], scalar1=CNT_LOCAL)
                nc.sync.dma_start(out=cc_in[li][m0:m0 + Mg, :], in_=sums[:])
            nc.gpsimd.collective_compute(
                "AllReduce", mybir.AluOpType.add,
                replica_groups=[list(range(N_CORES))],
                ins=[cc_in[li][:].opt()], outs=[cc_out[li][:].opt()])
            nL = LAYERS[li + 1]
            sts = []
            for kg in range(nL["n_kg"]):
                k0 = kg * 96
                sr = small.tile([96, 2], F32, tag="sr")
                nc.sync.dma_start(out=sr[:], in_=cc_out[li][k0:k0 + 96, :])
                gbt = gb_t[li][kg]
                mean = small.tile([96, 1], F32, tag="mean")
                nc.gpsimd.tensor_scalar_mul(
                    out=mean[:], in0=sr[:, 0:1], scalar1=1.0 / CNT_TOTAL)
                var = small.tile([96, 1], F32, tag="var")
                nc.gpsimd.tensor_scalar_mul(
                    out=var[:], in0=sr[:, 1:2], scalar1=1.0 / CNT_TOTAL)
                msq = small.tile([96, 1], F32, tag="msq")
                nc.gpsimd.tensor_mul(msq[:], mean[:], mean[:])
                nc.gpsimd.tensor_sub(var[:], var[:], msq[:])
                nc.gpsimd.tensor_scalar_add(
                    out=var[:], in0=var[:], scalar1=BN_EPS)
                std = small.tile([96, 1], F32, tag="std")
                nc.scalar.activation(
                    out=std[:], in_=var[:],
                    func=mybir.ActivationFunctionType.Sqrt)
                rstd = small.tile([96, 1], F32, tag="rstd")
                nc.vector.reciprocal(out=rstd[:], in_=std[:])
                st = stp.tile([96, 2], F32, tag=f"stv{kg}", name=f"st{li}_{kg}")
                nc.gpsimd.tensor_mul(st[:, 0:1], gbt[:, 0:1], rstd[:])
                tmp2 = small.tile([96, 1], F32, tag="tmp2")
                nc.gpsimd.tensor_mul(tmp2[:], mean[:], st[:, 0:1])
                nc.gpsimd.tensor_sub(st[:, 1:2], gbt[:, 1:2], tmp2[:])
                sts.append(st)
            if not nL.get("v3"):
                return sts
            # v3 consumer: stage per-channel st in DRAM, gather per-group
            # [128,2] tiles matching the (ky,c) partition layout
            for kg in range(nL["n_kg"]):
                nc.sync.dma_start(out=st_dram[li][kg * 96:(kg + 1) * 96, :],
                                  in_=sts[kg][:])
            G, rem, Cin = nL["G"], nL["rem"], nL["Cin"]
            stks = []
            for g in range(G):
                sk = stp.tile([128, 2], F32, tag=f"stk{g}",
                              name=f"stk{li}_{g}")
                for (p0, p1, ky, c0) in _dyruns(Cin, g, G, rem):
                    nc.sync.dma_start(out=sk[p0:p1, :],
                                      in_=st_dram[li][c0:c0 + (p1 - p0), :])
                stks.append(sk)
            return stks

        for li, L in enumerate(LAYERS):
            K, Mg, Mtot, Hout, Hin = L["K"], L["Mg"], L["Mtot"], L["Hout"], L["Hin"]
            n_kg_load = L["n_kg"]
            is_head = li == 4
            n_mm_kg = 1 if is_head else n_kg_load
            n_chunks = (Hout + R - 1) // R
            owned_chunks = OWN // R
            st_next = None

            # per-mg stats buffers [Mg, 64, 6]
            if not is_head:
                stat_t = [stats.tile([Mg, OWN, 6], F32, tag=f"sb{mg}",
                                     name=f"sb{li}_{mg}")
                          for mg in range(L["n_mg"])]

            for c in range(n_chunks):
                y0 = c * R
                rows = min(R, Hout - y0)
                used = (rows + 2) * WB  # data region size (from flat idx 1)
                tail = used + 1

                # ---- load input chunk per kgroup ----
                in_t = []
                if L.get("v3"):
                    # mixed (ky,c) packed input tiles: partition (ky,c) of
                    # group g holds channel c's rows shifted by ky-1, so only
                    # the 3 kx taps stream per group
                    G, rem, Cin = L["G"], L["rem"], L["Cin"]
                    used3 = rows * WB
                    src = y_dram[li - 1]
                    for g in range(G):
                        npg = 128 if g < G - 1 else rem
                        it = inpool.tile([128, R * WB + 2], DT, tag=f"in{g}")
                        nc.vector.memset(it[0:npg, 0:1], 0.0)
                        nc.vector.memset(it[0:npg, 1 + used3:2 + used3], 0.0)
                        zero_runs = []
                        for (p0, p1, ky, c0) in _dyruns(Cin, g, G, rem):
                            nch = p1 - p0
                            if y0 == 0 and ky == 0:
                                # global top edge: first row is zero pad
                                nc.vector.memset(it[p0:p1, 1:1 + WB], 0.0)
                                if rows > 1:
                                    nc.sync.dma_start(
                                        out=it[p0:p1, 1 + WB:1 + used3],
                                        in_=src[c0:c0 + nch, 0:rows - 1, :])
                                zero_runs.append((p0, p1))
                            else:
                                y0s = y0 + ky - 1
                                nc.sync.dma_start(
                                    out=it[p0:p1, 1:1 + used3],
                                    in_=src[c0:c0 + nch, y0s:y0s + rows, :])
                        nc.scalar.activation(
                            out=it[0:npg, 1:1 + used3],
                            in_=it[0:npg, 1:1 + used3],
                            func=mybir.ActivationFunctionType.Relu,
                            bias=st_cur[g][0:npg, 1:2],
                            scale=st_cur[g][0:npg, 0:1])
                        itv = it[0:npg, 1:1 + used3].rearrange(
                            "p (r w) -> p r w", w=WB)
                        nc.vector.memset(itv[:, :, 0:1], 0.0)
                        nc.vector.memset(itv[:, :, WB - 1:WB], 0.0)
                        for (p0, p1) in zero_runs:
                            # re-zero the edge row clobbered by normalize
                            nc.vector.memset(it[p0:p1, 1:1 + WB], 0.0)
                        in_t.append(it)
                for kg in range(0 if L.get("v3") else n_kg_load):
                    it = inpool.tile([K if li == 0 else 96, FLAT], DT,
                                     tag=f"in{kg}")
                    nc.vector.memset(it[:, 0:1], 0.0)
                    nc.vector.memset(it[:, tail:tail + 1], 0.0)
                    if li == 0:
                        ch0 = kg * 128
                        nc.sync.dma_start(
                            out=it[:, 1:1 + used],
                            in_=x_ext[ch0:ch0 + 128, y0:y0 + rows + 2, :])
                    else:
                        ch0 = kg * 96
                        src = y_dram[li - 1]
                        if y0 == 0:
                            nc.vector.memset(it[:, 1:1 + WB], 0.0)
                            nc.sync.dma_start(
                                out=it[:, 1 + WB:1 + used],
                                in_=src[ch0:ch0 + 96, 0:rows + 1, :])
                            na, nb = 1 + WB, 1 + used
                        else:
                            nc.sync.dma_start(
                                out=it[:, 1:1 + used],
                                in_=src[ch0:ch0 + 96, y0 - 1:y0 + rows + 1, :])
                            na, nb = 1, 1 + used
                        # normalize + relu (BN of previous layer), in place
                        nc.scalar.activation(
                            out=it[:, na:nb], in_=it[:, na:nb],
                            func=mybir.ActivationFunctionType.Relu,
                            bias=st_cur[kg][:, 1:2], scale=st_cur[kg][:, 0:1])
                        # zero the W pad columns (post-normalize)
                        v3 = it[:, 1:1 + used].rearrange(
                            "p (r w) -> p r w", w=WB)
                        nc.vector.memset(v3[:, :, 0:1], 0.0)
                        nc.vector.memset(v3[:, :, WB - 1:WB], 0.0)
                    in_t.append(it)

                # ---- head: magnitude sqrt(re^2+im^2) ----
                if is_head:
                    mag = inpool.tile([96, FLAT], DT, tag="in2")
                    lim = tail + 1
                    nc.vector.tensor_mul(mag[:, 0:lim], in_t[0][:, 0:lim],
                                         in_t[0][:, 0:lim])
                    # square imag in place (it has no further readers)
                    nc.vector.tensor_mul(in_t[1][:, 0:lim], in_t[1][:, 0:lim],
                                         in_t[1][:, 0:lim])
                    nc.vector.tensor_add(mag[:, 0:lim], mag[:, 0:lim],
                                         in_t[1][:, 0:lim])
                    nc.scalar.activation(
                        out=mag[:, 0:lim], in_=mag[:, 0:lim],
                        func=mybir.ActivationFunctionType.Sqrt)
                    mm_in = [mag]
                else:
                    mm_in = in_t

                # ---- matmul tiles: 2 output rows per PSUM tile ----
                n_t = (rows + 1) // 2
                if L.get("v3"):
                    # group-outer order in 4-tile halves: Scalar normalizes
                    # group g+1 while Tensor streams group g; halves keep 4
                    # PSUM banks free for the other mgroup's evictions
                    G, rem = L["G"], L["rem"]
                    for mg in range(L["n_mg"]):
                        m0 = mg * Mg
                        stg = stpool.tile([Mg, R * WB], DT, tag="st")
                        for j0 in range(0, n_t, 4):
                            j1 = min(n_t, j0 + 4)
                            pss = [pspool.tile(
                                [Mg, min(2, rows - 2 * j) * WB], F32,
                                tag="ps", name=f"ps{li}_{c}_{mg}_{j}")
                                for j in range(j0, j1)]
                            for g in range(G):
                                npg = 128 if g < G - 1 else rem
                                for dx in range(3):
                                    for idx, j in enumerate(range(j0, j1)):
                                        r2 = min(2, rows - 2 * j)
                                        off = 2 * j * WB + dx
                                        nc.tensor.matmul(
                                            pss[idx][:],
                                            w_t[li][0:npg, dx, g,
                                                    m0:m0 + Mg],
                                            in_t[g][0:npg,
                                                    off:off + r2 * WB],
                                            start=(g == 0 and dx == 0),
                                            stop=(g == G - 1 and dx == 2))
                            for idx, j in enumerate(range(j0, j1)):
                                r2 = min(2, rows - 2 * j)
                                N = r2 * WB
                                dst = stg[:, 2 * j * WB:2 * j * WB + N]
                                nc.vector.tensor_copy(out=dst, in_=pss[idx][:])
                                if y0 < OWN:
                                    slot = y0 + 2 * j
                                    dv = dst.rearrange("p (r w) -> p r w",
                                                       w=WB)
                                    for r in range(r2):
                                        nc.vector.bn_stats(
                                            out=stat_t[mg][:, slot + r:
                                                           slot + r + 1, :],
                                            in_=dv[:, r:r + 1, 1:1 + W])
                        nc.sync.dma_start(
                            out=y_dram[li][m0:m0 + Mg, y0:y0 + rows, :],
                            in_=stg[:, 0:rows * WB])
                for mg in range(0 if L.get("v3") else L["n_mg"]):
                    m0 = mg * Mg
                    stg = stpool.tile([Mg, R * WB], F32 if is_head else DT,
                                      tag="st")
                    for j in range(n_t):
                        r2 = min(2, rows - 2 * j)
                        N = r2 * WB
                        ps = pspool.tile([Mg, N], F32, tag="ps")
                        nmm = 9 * n_mm_kg
                        i_mm = 0
                        for kg in range(n_mm_kg):
                            for t in range(9):
                                dy, dx = t // 3 - 1, t % 3 - 1
                                off = 1 + (2 * j + 1 + dy) * WB + dx
                                nc.tensor.matmul(
                                    ps[:],
                                    w_t[li][:, t, kg, m0:m0 + Mg],
                                    mm_in[kg][:, off:off + N],
                                    start=(i_mm == 0), stop=(i_mm == nmm - 1))
                                i_mm += 1
                        dst = stg[:, 2 * j * WB:2 * j * WB + N]
                        if is_head:
                            nc.vector.tensor_scalar_add(
                                out=dst, in0=ps[:], scalar1=hb_t[:])
                        else:
                            nc.vector.tensor_copy(out=dst, in_=ps[:])
                            if y0 < OWN:
                                # stats on the bf16 values the next layer
                                # will actually read (2x DVE rate vs f32
                                # psum); walrus requires 6 elems/partition
                                # out, so one op per row
                                slot = y0 + 2 * j
                                dv = dst.rearrange("p (r w) -> p r w", w=WB)
                                for r in range(r2):
                                    nc.vector.bn_stats(
                                        out=stat_t[mg][:, slot + r:
                                                       slot + r + 1, :],
                                        in_=dv[:, r:r + 1, 1:1 + W])
                    if is_head:
                        nc.scalar.activation(
                            out=stg[0:1, 0:rows * WB], in_=stg[0:1, 0:rows * WB],
                            func=mybir.ActivationFunctionType.Sigmoid)
                        sv = stg[:, 0:rows * WB].rearrange(
                            "p (r w) -> p r w", w=WB)
                        nc.sync.dma_start(
                            out=out_ext[:, y0:y0 + rows, :],
                            in_=sv[:, :, 1:1 + W])
                    else:
                        nc.sync.dma_start(
                            out=y_dram[li][m0:m0 + Mg, y0:y0 + rows, :],
                            in_=stg[:, 0:rows * WB])

                if li == 0 and c == 0:
                    nc.sync.dma_start(out=cc_win[:], in_=eps_t[0:1, :])
                    nc.gpsimd.collective_compute(
                        "AllReduce", mybir.AluOpType.add,
                        replica_groups=[list(range(N_CORES))],
                        ins=[cc_win[:].opt()], outs=[cc_wout[:].opt()])

                # all owned rows done -> kick stats AllReduce; the halo
                # chunk(s) below overlap the collective latency
                if c == owned_chunks - 1 and not is_head:
                    st_next = emit_boundary(li, L, stat_t)

            if not is_head:
                st_cur = st_next

    nc.compile()
    _nc_cache[mode] = nc
    return nc


def _prep_inputs(x, w1r, w1i, g1, b1, w2r, w2i, g2, b2,
                 w3r, w3i, g3, b3, w4r, w4i, g4, b4, wc, bc, wg, bg,
                 mode):
    """Host-side shard + pack. Returns in_maps list of 8 dicts."""
    npdt = _npdt(mode)
    x = np.asarray(x, np.float32)

    # stacked block weights [Mtot, Cin, 3, 3]
    W1 = np.concatenate([w1r, w1i], axis=0)
    def blk(wr, wi):
        top = np.concatenate([wr, -wi], axis=1)
        bot = np.concatenate([wi, wr], axis=1)
        return np.concatenate([top, bot], axis=0)
    W2, W3, W4 = blk(w2r, w2i), blk(w3r, w3i), blk(w4r, w4i)
    W5 = np.concatenate([wc, wg], axis=0)
    Ws = [W1, W2, W3, W4, W5]

    def pack_w(Wf, K, nkg, flip):
        # -> [K, 9, nkg, Mtot] with t = ky*3+kx, k-groups along Cin
        if flip:
            Wf = Wf[:, :, ::-1, :]
        Mtot, Cin = Wf.shape[0], Wf.shape[1]
        a = Wf.transpose(2, 3, 1, 0).reshape(9, Cin, Mtot)  # [t, cin, m]
        a = a.reshape(9, nkg, K, Mtot).transpose(2, 0, 1, 3)  # [K,9,g,M]
        return np.ascontiguousarray(a, dtype=npdt)

    def pack_w_v3(Wf, G, flip):
        # -> [128, 3, G, Mtot]: contraction f = ky*Cin + c packed into
        # full 128-partition groups; the 3 kx taps stream
        if flip:
            Wf = Wf[:, :, ::-1, :]
        Mtot, Cin = Wf.shape[0], Wf.shape[1]
        KT = 3 * Cin
        a = Wf.transpose(2, 1, 3, 0).reshape(KT, 3, Mtot)  # [ky*Cin+c,kx,m]
        pad = G * 128 - KT
        if pad:
            a = np.concatenate(
                [a, np.zeros((pad, 3, Mtot), a.dtype)], axis=0)
        a = a.reshape(G, 128, 3, Mtot).transpose(1, 2, 0, 3)
        return np.ascontiguousarray(a, dtype=npdt)

    gbs = []
    for g, b in ((g1, b1), (g2, b2), (g3, b3), (g4, b4)):
        gs = np.concatenate([g, g]).astype(np.float32)
        bs = np.concatenate([b, b]).astype(np.float32)
        gbs.append(np.ascontiguousarray(np.stack([gs, bs], axis=1)))
    hb = np.concatenate([bc, bg]).astype(np.float32).reshape(3, 1)

    in_maps = []
    for core in range(N_CORES):
        b_idx, h = core // 2, core % 2
        xi = x[b_idx]
        if h == 1:
            xi = xi[:, ::-1, :]
        # x_shard [256, 70, WB]: row 0 zero (local -1), rows 1..69 = local 0..68
        xs = np.zeros((256, 70, WB), np.float32)
        xs[:, 1:70, 1:1 + W] = xi[:, 0:69, :]
        m = {"x": xs.astype(npdt), "hb": hb}
        for li, L in enumerate(LAYERS):
            if L.get("v3"):
                m[f"w{li + 1}"] = pack_w_v3(Ws[li], L["G"], flip=(h == 1))
            else:
                nkg = L["n_kg"] if li != 4 else 1
                m[f"w{li + 1}"] = pack_w(Ws[li], L["K"], nkg, flip=(h == 1))
        for li in range(4):
            m[f"gb{li + 1}"] = gbs[li]
        in_maps.append(m)
    return in_maps


_runner_cache = {}


def _get_runner(mode):
    """Build the SPMD jit executable once; returns run(in_maps) -> list of
    per-core output dicts. Mirrors bass2jax.run_bass_via_pjrt but caches the
    jitted callable so repeated kernel() calls don't re-trace/re-compile."""
    if mode in _runner_cache:
        return _runner_cache[mode]
    import jax
    from concourse import bass2jax
    from jax.experimental.shard_map import shard_map
    from jax.sharding import Mesh, PartitionSpec

    nc = build_program(mode)
    bass2jax.install_neuronx_cc_hook()

    partition_name = (nc.partition_id_tensor.name
                      if nc.partition_id_tensor else None)
    in_names, out_names, out_avals, zero_outs = [], [], [], []
    for alloc in nc.m.functions[0].allocations:
        if not isinstance(alloc, mybir.MemoryLocationSet):
            continue
        name = alloc.memorylocations[0].name
        if alloc.kind == "ExternalInput":
            if name != partition_name:
                in_names.append(name)
        elif alloc.kind == "ExternalOutput":
            shape = tuple(alloc.tensor_shape)
            dtype = mybir.dt.np(alloc.dtype)
            out_names.append(name)
            out_avals.append(jax.core.ShapedArray(shape, dtype))
            zero_outs.append(np.zeros(shape, dtype))
    n_params, n_outs = len(in_names), len(out_avals)
    all_names = list(in_names + out_names)
    if partition_name is not None:
        all_names.append(partition_name)
    all_names = tuple(all_names)
    donate = tuple(range(n_params, n_params + n_outs))

    def _body(*args):
        operands = list(args)
        if partition_name is not None:
            operands.append(bass2jax.partition_id_tensor())
        outs = bass2jax._bass_exec_p.bind(
            *operands,
            out_avals=tuple(out_avals),
            in_names=all_names,
            out_names=tuple(out_names),
            lowering_input_output_aliases=(),
            sim_require_finite=True,
            sim_require_nnan=True,
            nc=nc,
        )
        return tuple(outs)

    devices = jax.devices()[:N_CORES]
    mesh = Mesh(np.asarray(devices), ("core",))
    in_specs = (PartitionSpec("core"),) * (n_params + n_outs)
    out_specs = (PartitionSpec("core"),) * n_outs
    sharded = jax.jit(
        shard_map(_body, mesh=mesh, in_specs=in_specs, out_specs=out_specs,
                  check_rep=False),
        donate_argnums=donate, keep_unused=True)

    def run(in_maps):
        concat_in = [
            np.concatenate([np.asarray(in_maps[c][nm]) for c in
                            range(N_CORES)], axis=0)
            for nm in in_names
        ]
        concat_zeros = [
            np.zeros((N_CORES * z.shape[0], *z.shape[1:]), z.dtype)
            for z in zero_outs
        ]
        out_arrs = sharded(*concat_in, *concat_zeros)
        return [
            {nm: np.asarray(out_arrs[i]).reshape(N_CORES, *out_avals[i].shape)[c]
             for i, nm in enumerate(out_names)}
            for c in range(N_CORES)
        ]

    def time_device(in_maps, reps=5):
        """Time executions with inputs pre-staged on device (excludes host
        prep and host->device transfer). Returns list of seconds."""
        import time as _time
        from jax.sharding import NamedSharding
        concat_in = [
            np.concatenate([np.asarray(in_maps[c][nm]) for c in
                            range(N_CORES)], axis=0)
            for nm in in_names
        ]
        sh = NamedSharding(mesh, PartitionSpec("core"))
        dev_in = [jax.device_put(a, sh) for a in concat_in]
        for a in dev_in:
            a.block_until_ready()
        times = []
        for _ in range(reps):
            concat_zeros = [
                jax.device_put(
                    np.zeros((N_CORES * z.shape[0], *z.shape[1:]), z.dtype),
                    sh)
                for z in zero_outs
            ]
            for a in concat_zeros:
                a.block_until_ready()
            t0 = _time.time()
            out_arrs = sharded(*dev_in, *concat_zeros)
            for o in out_arrs:
                o.block_until_ready()
            times.append(_time.time() - t0)
        return times

    run.time_device = time_device
    _runner_cache[mode] = run
    return run


def kernel(**inputs):
    mode = DT_MODE
    run = _get_runner(mode)
    in_maps = _prep_inputs(mode=mode, **inputs)
    results = run(in_maps)
    out = np.zeros((4, 3, H, W), np.float32)
    for core in range(N_CORES):
        b_idx, h = core // 2, core % 2
        oc = results[core]["out"]  # [3, 64, W]
        if h == 0:
            out[b_idx, :, 0:OWN, :] = oc
        else:
            out[b_idx, :, OWN:H, :] = oc[:, ::-1, :]
    return out



# revision 23
# speedup vs baseline: 51.5143x; 1.0057x over previous
"""Trainium2 Bass kernel for nn_ComplexFFTRadNet (complex CNN, 4 cconv+BN+ReLU
layers, |.| magnitude, two 3x3 conv heads, sigmoid on cls channel).

Sharding: 8 NeuronCores = batch(4) x H-halves(2). Each core computes 64 output
rows of one image. Bottom halves are vertically FLIPPED on the host (input rows
and conv-kernel dy both reversed) so that every core runs the identical SPMD
program: global image edge at local top, 5 rows of neighbor halo at local
bottom. BatchNorm statistics (training-style, over N,H,W) are computed locally
per channel with bn_stats/bn_aggr over each core's owned 64 rows and combined
with a tiny AllReduce per layer.

Convolution = 9 shifted-window matmuls accumulated in PSUM; channels on the
partition (contraction) axis; activations stored row-major [C, H, W+2] with
zero pad columns so all 9 taps are contiguous-offset reads of one SBUF tile.
Real/imag are stacked along channels, so a complex conv is one real conv with
the block weight matrix [[wr, -wi], [wi, wr]].
"""
import os
import sys
import numpy as np
from contextlib import ExitStack

sys.path.insert(0, "/opt/trn_rl_repo")

from concourse import bass, bass_utils, tile, mybir, bacc  # noqa: E402

try:
    import ml_dtypes
    _BF16 = ml_dtypes.bfloat16
except Exception:  # pragma: no cover
    _BF16 = None

N_CORES = 8
H, W = 128, 224
WB = W + 2          # padded width
OWN = 64            # owned rows per core
R = 8               # output rows per chunk
CNT_LOCAL = float(OWN * W)          # elements per channel per core
CNT_TOTAL = float(4 * H * W)        # elements per channel globally
BN_EPS = 1e-5

# matmul/storage dtype: "f32" (exact) or "bf16" (4x faster PE, ~0.5% err)
DT_MODE = os.environ.get("KERNEL_DT", "bf16")

# layer table: (n_kg_load, kg_ch, K, n_kg_mm, n_mg, M_total, H_in_data, H_out)
#   L1: x[256] -> stacked 288 (yr144,yi144), kgroups 2x128, mgroups 3x96
#   L2-L4: v3 mixed (ky,c) K-packing: contraction space 3*Cin flattened as
#     f = ky*Cin + c into full 128-partition groups; only the 3 kx taps
#     stream, cutting matmul pairs per 2-row tile (L2 27->21, L3/L4 18->15)
#   L5 head: mag[96] -> 3
LAYERS = [
    dict(K=128, n_kg=2, Cin=256, n_mg=3, Mg=96, Mtot=288, Hin=69, Hout=68),
    dict(K=96, n_kg=3, Cin=288, n_mg=2, Mg=96, Mtot=192, Hin=68, Hout=67,
         v3=True),
    dict(K=96, n_kg=2, Cin=192, n_mg=2, Mg=96, Mtot=192, Hin=67, Hout=66,
         v3=True),
    dict(K=96, n_kg=2, Cin=192, n_mg=2, Mg=96, Mtot=192, Hin=66, Hout=65,
         v3=True),
    dict(K=96, n_kg=2, Cin=192, n_mg=1, Mg=3, Mtot=3, Hin=65, Hout=64),
]
for _L in LAYERS:
    if _L.get("v3"):
        _KT = 3 * _L["Cin"]
        _L["G"] = (_KT + 127) // 128
        _L["rem"] = _KT - (_L["G"] - 1) * 128


def _dyruns(Cin, g, G, rem):
    """Partition runs of group g in the flattened (ky, c) space.

    Returns (p0, p1, ky, c0): partitions [p0,p1) of the group hold channels
    c0.. with row-tap ky (0/1/2 = input row offset ky-1)."""
    f0 = g * 128
    f1 = f0 + (128 if g < G - 1 else rem)
    runs = []
    f = f0
    while f < f1:
        ky, c0 = f // Cin, f % Cin
        fend = min(f1, (ky + 1) * Cin)
        runs.append((f - f0, fend - f0, ky, c0))
        f = fend
    return runs

_nc_cache = {}


def _dt(mode):
    return mybir.dt.bfloat16 if mode == "bf16" else mybir.dt.float32


def _npdt(mode):
    return _BF16 if mode == "bf16" else np.float32


def build_program(mode):
    if mode in _nc_cache:
        return _nc_cache[mode]
    DT = _dt(mode)
    F32 = mybir.dt.float32
    R = 16 if mode == "bf16" else 8  # chunk rows (SBUF-limited for f32)
    nc = bacc.Bacc("TRN2", target_bir_lowering=False, debug=False,
                   num_devices=N_CORES)

    # ---- external I/O ----
    x_ext = nc.dram_tensor("x", [256, 70, WB], DT, kind="ExternalInput").ap()
    w_ext = []
    for li, L in enumerate(LAYERS):
        if L.get("v3"):
            shape = [128, 3, L["G"], L["Mtot"]]
        else:
            shape = [L["K"], 9, L["n_kg"] if li != 4 else 1, L["Mtot"]]
        w_ext.append(nc.dram_tensor(
            f"w{li + 1}", shape, DT, kind="ExternalInput").ap())
    gb_ext = []
    for li in range(4):
        gb_ext.append(nc.dram_tensor(
            f"gb{li + 1}", [LAYERS[li]["Mtot"], 2], F32,
            kind="ExternalInput").ap())
    hb_ext = nc.dram_tensor("hb", [3, 1], F32, kind="ExternalInput").ap()
    out_ext = nc.dram_tensor("out", [3, OWN, W], F32,
                             kind="ExternalOutput").ap()

    FLAT = (R + 2) * WB + 2  # flat in-tile size (1 lead + rows+2 + 1 tail)

    with tile.TileContext(nc) as tc, ExitStack() as ctx:
        wpool = ctx.enter_context(tc.tile_pool(name="wts", bufs=2))
        inpool = ctx.enter_context(tc.tile_pool(name="inp", bufs=2))
        stpool = ctx.enter_context(tc.tile_pool(name="stage", bufs=3))
        pspool = ctx.enter_context(tc.tile_pool(name="ps", bufs=8, space="PSUM"))
        stats = ctx.enter_context(tc.tile_pool(name="stats", bufs=1))
        small = ctx.enter_context(tc.tile_pool(name="small", bufs=4))
        stp = ctx.enter_context(tc.tile_pool(name="stv", bufs=2))
        dram = ctx.enter_context(tc.tile_pool(name="dram", bufs=1, space="DRAM"))

        # constants
        eps_t = small.tile([128, 1], F32, tag="eps")
        nc.vector.memset(eps_t[:], BN_EPS)
        hb_t = small.tile([3, 1], F32, tag="hb")
        nc.sync.dma_start(out=hb_t[:], in_=hb_ext)

        # per-layer weights resident whole kernel; w1 first (startup path)
        w_t = []
        for li, L in enumerate(LAYERS):
            if L.get("v3"):
                t = wpool.tile([128, 3, L["G"], L["Mtot"]], DT, tag="w",
                               name=f"wt{li}")
            else:
                nkg = L["n_kg"] if li != 4 else 1
                t = wpool.tile([L["K"], 9, nkg, L["Mtot"]], DT, tag="w",
                               name=f"wt{li}")
            nc.sync.dma_start(out=t[:], in_=w_ext[li])
            w_t.append(t)

        # preload all BN gamma/beta slices (per consumer kgroup) upfront
        gb_t = []  # gb_t[li][kg] -> [96,2] f32
        for li in range(4):
            nkg_next = LAYERS[li + 1]["n_kg"]
            tiles = []
            for kg in range(nkg_next):
                k0 = kg * 96
                g = small.tile([96, 2], F32, tag=f"gb{li}_{kg}",
                               name=f"gbt{li}_{kg}")
                nc.sync.dma_start(out=g[:], in_=gb_ext[li][k0:k0 + 96, :])
                tiles.append(g)
            gb_t.append(tiles)

        # DRAM spill buffers for layer outputs (raw conv out, pre-BN)
        y_dram = []
        for li in range(4):
            L = LAYERS[li]
            y_dram.append(dram.tile([L["Mtot"], L["Hout"], WB], DT,
                                    tag=f"y{li}", name=f"y{li}"))
        cc_in = [dram.tile([LAYERS[li]["Mtot"], 2], F32, tag=f"cci{li}",
                           name=f"cci{li}")
                 for li in range(4)]
        # Shared address space: the HBM-HBM AllReduce writes peers directly
        cc_out = [nc.dram_tensor(f"cco{li}", [LAYERS[li]["Mtot"], 2], F32,
                                 kind="Internal", addr_space="Shared").ap()
                  for li in range(4)]
        # tiny warm-up collective: absorbs cross-core NEFF-start skew during
        # L1 compute so the first real BN AllReduce doesn't pay it
        cc_win = dram.tile([1, 1], F32, tag="ccw", name="ccw")
        cc_wout = nc.dram_tensor("ccw_out", [1, 1], F32,
                                 kind="Internal", addr_space="Shared").ap()
        # per-channel scale/shift staged via DRAM so v3 consumers can gather
        # them into the mixed (ky,c) partition layout
        st_dram = [dram.tile([LAYERS[li + 1]["Cin"], 2], F32, tag=f"std{li}",
                             name=f"std{li}")
                   for li in range(3)]

        st_cur = None  # list per kgroup of [96,2] tiles (s=col0, t=col1)

        def emit_boundary(li, L, stat_t):
            """BN aggregate + AllReduce + next-layer scale/shift.

            Emitted right after the last owned-row chunk so the collective
            overlaps the halo-row matmuls. Small arithmetic runs on the idle
            GpSimd queue; only the sqrt needs the Scalar engine."""
            Mg = L["Mg"]
            for mg in range(L["n_mg"]):
                m0 = mg * Mg
                mv = small.tile([Mg, 2], F32, tag="mv")
                nc.vector.bn_aggr(out=mv[:], in_=stat_t[mg][:])
                sums = small.tile([Mg, 2], F32, tag="sums")
                nc.vector.tensor_scalar_mul(
                    out=sums[:, 0:1], in0=mv[:, 0:1], scalar1=CNT_LOCAL)
                sq = small.tile([Mg, 1], F32, tag="sq")
                nc.vector.tensor_mul(sq[:], mv[:, 0:1], mv[:, 0:1])
                nc.vector.tensor_add(sq[:], sq[:], mv[:, 1:2])
                nc.vector.tensor_scalar_mul(
                    out=sums[:, 1:2], in0=sq[:], scalar1=CNT_LOCAL)
                nc.sync.dma_start(out=cc_in[li][m0:m0 + Mg, :], in_=sums[:])
            nc.gpsimd.collective_compute(
                "AllReduce", mybir.AluOpType.add,
                replica_groups=[list(range(N_CORES))],
                ins=[cc_in[li][:].opt()], outs=[cc_out[li][:].opt()])
            nL = LAYERS[li + 1]
            sts = []
            for kg in range(nL["n_kg"]):
                k0 = kg * 96
                sr = small.tile([96, 2], F32, tag="sr")
                nc.sync.dma_start(out=sr[:], in_=cc_out[li][k0:k0 + 96, :])
                gbt = gb_t[li][kg]
                mean = small.tile([96, 1], F32, tag="mean")
                nc.gpsimd.tensor_scalar_mul(
                    out=mean[:], in0=sr[:, 0:1], scalar1=1.0 / CNT_TOTAL)
                var = small.tile([96, 1], F32, tag="var")
                nc.gpsimd.tensor_scalar_mul(
                    out=var[:], in0=sr[:, 1:2], scalar1=1.0 / CNT_TOTAL)
                msq = small.tile([96, 1], F32, tag="msq")
                nc.gpsimd.tensor_mul(msq[:], mean[:], mean[:])
                nc.gpsimd.tensor_sub(var[:], var[:], msq[:])
                nc.gpsimd.tensor_scalar_add(
                    out=var[:], in0=var[:], scalar1=BN_EPS)
                std = small.tile([96, 1], F32, tag="std")
                nc.scalar.activation(
                    out=std[:], in_=var[:],
                    func=mybir.ActivationFunctionType.Sqrt)
                rstd = small.tile([96, 1], F32, tag="rstd")
                nc.vector.reciprocal(out=rstd[:], in_=std[:])
                st = stp.tile([96, 2], F32, tag=f"stv{kg}", name=f"st{li}_{kg}")
                nc.gpsimd.tensor_mul(st[:, 0:1], gbt[:, 0:1], rstd[:])
                tmp2 = small.tile([96, 1], F32, tag="tmp2")
                nc.gpsimd.tensor_mul(tmp2[:], mean[:], st[:, 0:1])
                nc.gpsimd.tensor_sub(st[:, 1:2], gbt[:, 1:2], tmp2[:])
                sts.append(st)
            if not nL.get("v3"):
                return sts
            # v3 consumer: stage per-channel st in DRAM, gather per-group
            # [128,2] tiles matching the (ky,c) partition layout
            for kg in range(nL["n_kg"]):
                nc.sync.dma_start(out=st_dram[li][kg * 96:(kg + 1) * 96, :],
                                  in_=sts[kg][:])
            G, rem, Cin = nL["G"], nL["rem"], nL["Cin"]
            stks = []
            for g in range(G):
                sk = stp.tile([128, 2], F32, tag=f"stk{g}",
                              name=f"stk{li}_{g}")
                for (p0, p1, ky, c0) in _dyruns(Cin, g, G, rem):
                    nc.sync.dma_start(out=sk[p0:p1, :],
                                      in_=st_dram[li][c0:c0 + (p1 - p0), :])
                stks.append(sk)
            return stks

        for li, L in enumerate(LAYERS):
            K, Mg, Mtot, Hout, Hin = L["K"], L["Mg"], L["Mtot"], L["Hout"], L["Hin"]
            n_kg_load = L["n_kg"]
            is_head = li == 4
            n_mm_kg = 1 if is_head else n_kg_load
            n_chunks = (Hout + R - 1) // R
            owned_chunks = OWN // R
            st_next = None

            # per-mg stats buffers [Mg, 64, 6]
            if not is_head:
                stat_t = [stats.tile([Mg, OWN, 6], F32, tag=f"sb{mg}",
                                     name=f"sb{li}_{mg}")
                          for mg in range(L["n_mg"])]

            for c in range(n_chunks):
                y0 = c * R
                rows = min(R, Hout - y0)
                used = (rows + 2) * WB  # data region size (from flat idx 1)
                tail = used + 1

                # ---- load input chunk per kgroup ----
                in_t = []
                if L.get("v3"):
                    # mixed (ky,c) packed input tiles: partition (ky,c) of
                    # group g holds channel c's rows shifted by ky-1, so only
                    # the 3 kx taps stream per group
                    G, rem, Cin = L["G"], L["rem"], L["Cin"]
                    used3 = rows * WB
                    src = y_dram[li - 1]
                    for g in range(G):
                        npg = 128 if g < G - 1 else rem
                        it = inpool.tile([128, R * WB + 2], DT, tag=f"in{g}")
                        nc.vector.memset(it[0:npg, 0:1], 0.0)
                        nc.vector.memset(it[0:npg, 1 + used3:2 + used3], 0.0)
                        zero_runs = []
                        for (p0, p1, ky, c0) in _dyruns(Cin, g, G, rem):
                            nch = p1 - p0
                            if y0 == 0 and ky == 0:
                                # global top edge: first row is zero pad
                                nc.vector.memset(it[p0:p1, 1:1 + WB], 0.0)
                                if rows > 1:
                                    nc.sync.dma_start(
                                        out=it[p0:p1, 1 + WB:1 + used3],
                                        in_=src[c0:c0 + nch, 0:rows - 1, :])
                                zero_runs.append((p0, p1))
                            else:
                                y0s = y0 + ky - 1
                                nc.sync.dma_start(
                                    out=it[p0:p1, 1:1 + used3],
                                    in_=src[c0:c0 + nch, y0s:y0s + rows, :])
                        nc.scalar.activation(
                            out=it[0:npg, 1:1 + used3],
                            in_=it[0:npg, 1:1 + used3],
                            func=mybir.ActivationFunctionType.Relu,
                            bias=st_cur[g][0:npg, 1:2],
                            scale=st_cur[g][0:npg, 0:1])
                        itv = it[0:npg, 1:1 + used3].rearrange(
                            "p (r w) -> p r w", w=WB)
                        nc.vector.memset(itv[:, :, 0:1], 0.0)
                        nc.vector.memset(itv[:, :, WB - 1:WB], 0.0)
                        for (p0, p1) in zero_runs:
                            # re-zero the edge row clobbered by normalize
                            nc.vector.memset(it[p0:p1, 1:1 + WB], 0.0)
                        in_t.append(it)
                for kg in range(0 if L.get("v3") else n_kg_load):
                    it = inpool.tile([K if li == 0 else 96, FLAT], DT,
                                     tag=f"in{kg}")
                    nc.vector.memset(it[:, 0:1], 0.0)
                    nc.vector.memset(it[:, tail:tail + 1], 0.0)
                    if li == 0:
                        ch0 = kg * 128
                        nc.sync.dma_start(
                            out=it[:, 1:1 + used],
                            in_=x_ext[ch0:ch0 + 128, y0:y0 + rows + 2, :])
                    else:
                        ch0 = kg * 96
                        src = y_dram[li - 1]
                        if y0 == 0:
                            nc.vector.memset(it[:, 1:1 + WB], 0.0)
                            nc.sync.dma_start(
                                out=it[:, 1 + WB:1 + used],
                                in_=src[ch0:ch0 + 96, 0:rows + 1, :])
                            na, nb = 1 + WB, 1 + used
                        else:
                            nc.sync.dma_start(
                                out=it[:, 1:1 + used],
                                in_=src[ch0:ch0 + 96, y0 - 1:y0 + rows + 1, :])
                            na, nb = 1, 1 + used
                        # normalize + relu (BN of previous layer), in place
                        nc.scalar.activation(
                            out=it[:, na:nb], in_=it[:, na:nb],
                            func=mybir.ActivationFunctionType.Relu,
                            bias=st_cur[kg][:, 1:2], scale=st_cur[kg][:, 0:1])
                        # zero the W pad columns (post-normalize)
                        v3 = it[:, 1:1 + used].rearrange(
                            "p (r w) -> p r w", w=WB)
                        nc.vector.memset(v3[:, :, 0:1], 0.0)
                        nc.vector.memset(v3[:, :, WB - 1:WB], 0.0)
                    in_t.append(it)

                # ---- head: magnitude sqrt(re^2+im^2) ----
                if is_head:
                    mag = inpool.tile([96, FLAT], DT, tag="in2")
                    lim = tail + 1
                    nc.vector.tensor_mul(mag[:, 0:lim], in_t[0][:, 0:lim],
                                         in_t[0][:, 0:lim])
                    # square imag in place (it has no further readers)
                    nc.vector.tensor_mul(in_t[1][:, 0:lim], in_t[1][:, 0:lim],
                                         in_t[1][:, 0:lim])
                    nc.vector.tensor_add(mag[:, 0:lim], mag[:, 0:lim],
                                         in_t[1][:, 0:lim])
                    nc.scalar.activation(
                        out=mag[:, 0:lim], in_=mag[:, 0:lim],
                        func=mybir.ActivationFunctionType.Sqrt)
                    mm_in = [mag]
                else:
                    mm_in = in_t

                # ---- matmul tiles: 2 output rows per PSUM tile ----
                n_t = (rows + 1) // 2
                if L.get("v3"):
                    # group-outer order in 4-tile halves: Scalar normalizes
                    # group g+1 while Tensor streams group g; halves keep 4
                    # PSUM banks free for the other mgroup's evictions
                    G, rem = L["G"], L["rem"]
                    for mg in range(L["n_mg"]):
                        m0 = mg * Mg
                        stg = stpool.tile([Mg, R * WB], DT, tag="st")
                        for j0 in range(0, n_t, 4):
                            j1 = min(n_t, j0 + 4)
                            pss = [pspool.tile(
                                [Mg, min(2, rows - 2 * j) * WB], F32,
                                tag="ps", name=f"ps{li}_{c}_{mg}_{j}")
                                for j in range(j0, j1)]
                            for g in range(G):
                                npg = 128 if g < G - 1 else rem
                                for dx in range(3):
                                    for idx, j in enumerate(range(j0, j1)):
                                        r2 = min(2, rows - 2 * j)
                                        off = 2 * j * WB + dx
                                        nc.tensor.matmul(
                                            pss[idx][:],
                                            w_t[li][0:npg, dx, g,
                                                    m0:m0 + Mg],
                                            in_t[g][0:npg,
                                                    off:off + r2 * WB],
                                            start=(g == 0 and dx == 0),
                                            stop=(g == G - 1 and dx == 2))
                            for idx, j in enumerate(range(j0, j1)):
                                r2 = min(2, rows - 2 * j)
                                N = r2 * WB
                                dst = stg[:, 2 * j * WB:2 * j * WB + N]
                                nc.vector.tensor_copy(out=dst, in_=pss[idx][:])
                                if y0 < OWN:
                                    slot = y0 + 2 * j
                                    dv = dst.rearrange("p (r w) -> p r w",
                                                       w=WB)
                                    for r in range(r2):
                                        nc.vector.bn_stats(
                                            out=stat_t[mg][:, slot + r:
                                                           slot + r + 1, :],
                                            in_=dv[:, r:r + 1, 1:1 + W])
                        nc.sync.dma_start(
                            out=y_dram[li][m0:m0 + Mg, y0:y0 + rows, :],
                            in_=stg[:, 0:rows * WB])
                for mg in range(0 if L.get("v3") else L["n_mg"]):
                    m0 = mg * Mg
                    stg = stpool.tile([Mg, R * WB], F32 if is_head else DT,
                                      tag="st")
                    for j in range(n_t):
                        r2 = min(2, rows - 2 * j)
                        N = r2 * WB
                        ps = pspool.tile([Mg, N], F32, tag="ps")
                        nmm = 9 * n_mm_kg
                        i_mm = 0
                        for kg in range(n_mm_kg):
                            for t in range(9):
                                dy, dx = t // 3 - 1, t % 3 - 1
                                off = 1 + (2 * j + 1 + dy) * WB + dx
                                nc.tensor.matmul(
                                    ps[:],
                                    w_t[li][:, t, kg, m0:m0 + Mg],
                                    mm_in[kg][:, off:off + N],
                                    start=(i_mm == 0), stop=(i_mm == nmm - 1))
                                i_mm += 1
                        dst = stg[:, 2 * j * WB:2 * j * WB + N]
                        if is_head:
                            nc.vector.tensor_scalar_add(
                                out=dst, in0=ps[:], scalar1=hb_t[:])
                        else:
                            nc.vector.tensor_copy(out=dst, in_=ps[:])
                            if y0 < OWN:
                                # stats on the bf16 values the next layer
                                # will actually read (2x DVE rate vs f32
                                # psum); walrus requires 6 elems/partition
                                # out, so one op per row
                                slot = y0 + 2 * j
                                dv = dst.rearrange("p (r w) -> p r w", w=WB)
                                for r in range(r2):
                                    nc.vector.bn_stats(
                                        out=stat_t[mg][:, slot + r:
                                                       slot + r + 1, :],
                                        in_=dv[:, r:r + 1, 1:1 + W])
                    if is_head:
                        nc.scalar.activation(
                            out=stg[0:1, 0:rows * WB], in_=stg[0:1, 0:rows * WB],
                            func=mybir.ActivationFunctionType.Sigmoid)
                        sv = stg[:, 0:rows * WB].rearrange(
                            "p (r w) -> p r w", w=WB)
                        nc.sync.dma_start(
                            out=out_ext[:, y0:y0 + rows, :],
                            in_=sv[:, :, 1:1 + W])
                    else:
                        nc.sync.dma_start(
                            out=y_dram[li][m0:m0 + Mg, y0:y0 + rows, :],
                            in_=stg[:, 0:rows * WB])

                if li == 0 and c == 0:
                    nc.sync.dma_start(out=cc_win[:], in_=eps_t[0:1, :])
                    nc.gpsimd.collective_compute(
                        "AllReduce", mybir.AluOpType.add,
                        replica_groups=[list(range(N_CORES))],
                        ins=[cc_win[:].opt()], outs=[cc_wout[:].opt()])

                # all owned rows done -> kick stats AllReduce; the halo
                # chunk(s) below overlap the collective latency
                if c == owned_chunks - 1 and not is_head:
                    st_next = emit_boundary(li, L, stat_t)

            if not is_head:
                st_cur = st_next

    nc.compile()
    _nc_cache[mode] = nc
    return nc


def _prep_inputs(x, w1r, w1i, g1, b1, w2r, w2i, g2, b2,
                 w3r, w3i, g3, b3, w4r, w4i, g4, b4, wc, bc, wg, bg,
                 mode):
    """Host-side shard + pack. Returns in_maps list of 8 dicts."""
    npdt = _npdt(mode)
    x = np.asarray(x, np.float32)

    # stacked block weights [Mtot, Cin, 3, 3]
    W1 = np.concatenate([w1r, w1i], axis=0)
    def blk(wr, wi):
        top = np.concatenate([wr, -wi], axis=1)
        bot = np.concatenate([wi, wr], axis=1)
        return np.concatenate([top, bot], axis=0)
    W2, W3, W4 = blk(w2r, w2i), blk(w3r, w3i), blk(w4r, w4i)
    W5 = np.concatenate([wc, wg], axis=0)
    Ws = [W1, W2, W3, W4, W5]

    def pack_w(Wf, K, nkg, flip):
        # -> [K, 9, nkg, Mtot] with t = ky*3+kx, k-groups along Cin
        if flip:
            Wf = Wf[:, :, ::-1, :]
        Mtot, Cin = Wf.shape[0], Wf.shape[1]
        a = Wf.transpose(2, 3, 1, 0).reshape(9, Cin, Mtot)  # [t, cin, m]
        a = a.reshape(9, nkg, K, Mtot).transpose(2, 0, 1, 3)  # [K,9,g,M]
        return np.ascontiguousarray(a, dtype=npdt)

    def pack_w_v3(Wf, G, flip):
        # -> [128, 3, G, Mtot]: contraction f = ky*Cin + c packed into
        # full 128-partition groups; the 3 kx taps stream
        if flip:
            Wf = Wf[:, :, ::-1, :]
        Mtot, Cin = Wf.shape[0], Wf.shape[1]
        KT = 3 * Cin
        a = Wf.transpose(2, 1, 3, 0).reshape(KT, 3, Mtot)  # [ky*Cin+c,kx,m]
        pad = G * 128 - KT
        if pad:
            a = np.concatenate(
                [a, np.zeros((pad, 3, Mtot), a.dtype)], axis=0)
        a = a.reshape(G, 128, 3, Mtot).transpose(1, 2, 0, 3)
        return np.ascontiguousarray(a, dtype=npdt)

    gbs = []
    for g, b in ((g1, b1), (g2, b2), (g3, b3), (g4, b4)):
        gs = np.concatenate([g, g]).astype(np.float32)
        bs = np.concatenate([b, b]).astype(np.float32)
        gbs.append(np.ascontiguousarray(np.stack([gs, bs], axis=1)))
    hb = np.concatenate([bc, bg]).astype(np.float32).reshape(3, 1)

    in_maps = []
    for core in range(N_CORES):
        b_idx, h = core // 2, core % 2
        xi = x[b_idx]
        if h == 1:
            xi = xi[:, ::-1, :]
        # x_shard [256, 70, WB]: row 0 zero (local -1), rows 1..69 = local 0..68
        xs = np.zeros((256, 70, WB), np.float32)
        xs[:, 1:70, 1:1 + W] = xi[:, 0:69, :]
        m = {"x": xs.astype(npdt), "hb": hb}
        for li, L in enumerate(LAYERS):
            if L.get("v3"):
                m[f"w{li + 1}"] = pack_w_v3(Ws[li], L["G"], flip=(h == 1))
            else:
                nkg = L["n_kg"] if li != 4 else 1
                m[f"w{li + 1}"] = pack_w(Ws[li], L["K"], nkg, flip=(h == 1))
        for li in range(4):
            m[f"gb{li + 1}"] = gbs[li]
        in_maps.append(m)
    return in_maps


_runner_cache = {}


def _get_runner(mode):
    """Build the SPMD jit executable once; returns run(in_maps) -> list of
    per-core output dicts. Mirrors bass2jax.run_bass_via_pjrt but caches the
    jitted callable so repeated kernel() calls don't re-trace/re-compile."""
    if mode in _runner_cache:
        return _runner_cache[mode]
    import jax
    from concourse import bass2jax
    from jax.experimental.shard_map import shard_map
    from jax.sharding import Mesh, PartitionSpec

    nc = build_program(mode)
    bass2jax.install_neuronx_cc_hook()

    partition_name = (nc.partition_id_tensor.name
                      if nc.partition_id_tensor else None)
    in_names, out_names, out_avals, zero_outs = [], [], [], []
    for alloc in nc.m.functions[0].allocations:
        if not isinstance(alloc, mybir.MemoryLocationSet):
            continue
        name = alloc.memorylocations[0].name
        if alloc.kind == "ExternalInput":
            if name != partition_name:
                in_names.append(name)
        elif alloc.kind == "ExternalOutput":
            shape = tuple(alloc.tensor_shape)
            dtype = mybir.dt.np(alloc.dtype)
            out_names.append(name)
            out_avals.append(jax.core.ShapedArray(shape, dtype))
            zero_outs.append(np.zeros(shape, dtype))
    n_params, n_outs = len(in_names), len(out_avals)
    all_names = list(in_names + out_names)
    if partition_name is not None:
        all_names.append(partition_name)
    all_names = tuple(all_names)
    donate = tuple(range(n_params, n_params + n_outs))

    def _body(*args):
        operands = list(args)
        if partition_name is not None:
            operands.append(bass2jax.partition_id_tensor())
        outs = bass2jax._bass_exec_p.bind(
            *operands,
            out_avals=tuple(out_avals),
            in_names=all_names,
            out_names=tuple(out_names),
            lowering_input_output_aliases=(),
            sim_require_finite=True,
            sim_require_nnan=True,
            nc=nc,
        )
        return tuple(outs)

    devices = jax.devices()[:N_CORES]
    mesh = Mesh(np.asarray(devices), ("core",))
    in_specs = (PartitionSpec("core"),) * (n_params + n_outs)
    out_specs = (PartitionSpec("core"),) * n_outs
    sharded = jax.jit(
        shard_map(_body, mesh=mesh, in_specs=in_specs, out_specs=out_specs,
                  check_rep=False),
        donate_argnums=donate, keep_unused=True)

    def run(in_maps):
        concat_in = [
            np.concatenate([np.asarray(in_maps[c][nm]) for c in
                            range(N_CORES)], axis=0)
            for nm in in_names
        ]
        concat_zeros = [
            np.zeros((N_CORES * z.shape[0], *z.shape[1:]), z.dtype)
            for z in zero_outs
        ]
        out_arrs = sharded(*concat_in, *concat_zeros)
        return [
            {nm: np.asarray(out_arrs[i]).reshape(N_CORES, *out_avals[i].shape)[c]
             for i, nm in enumerate(out_names)}
            for c in range(N_CORES)
        ]

    def time_device(in_maps, reps=5):
        """Time executions with inputs pre-staged on device (excludes host
        prep and host->device transfer). Returns list of seconds."""
        import time as _time
        from jax.sharding import NamedSharding
        concat_in = [
            np.concatenate([np.asarray(in_maps[c][nm]) for c in
                            range(N_CORES)], axis=0)
            for nm in in_names
        ]
        sh = NamedSharding(mesh, PartitionSpec("core"))
        dev_in = [jax.device_put(a, sh) for a in concat_in]
        for a in dev_in:
            a.block_until_ready()
        times = []
        for _ in range(reps):
            concat_zeros = [
                jax.device_put(
                    np.zeros((N_CORES * z.shape[0], *z.shape[1:]), z.dtype),
                    sh)
                for z in zero_outs
            ]
            for a in concat_zeros:
                a.block_until_ready()
            t0 = _time.time()
            out_arrs = sharded(*dev_in, *concat_zeros)
            for o in out_arrs:
                o.block_until_ready()
            times.append(_time.time() - t0)
        return times

    run.time_device = time_device
    _runner_cache[mode] = run
    return run


def kernel(**inputs):
    mode = DT_MODE
    run = _get_runner(mode)
    in_maps = _prep_inputs(mode=mode, **inputs)
    results = run(in_maps)
    out = np.zeros((4, 3, H, W), np.float32)
    for core in range(N_CORES):
        b_idx, h = core // 2, core % 2
        oc = results[core]["out"]  # [3, 64, W]
        if h == 0:
            out[b_idx, :, 0:OWN, :] = oc
        else:
            out[b_idx, :, OWN:H, :] = oc[:, ::-1, :]
    return out

